# revision 16
# baseline (speedup 1.0000x reference)
"""ARCAttention (MLA + pattern-attention + gate) distributed Bass kernel for 8 TRN2 NeuronCores.

Sharding: data-parallel over batch (B=2) x tensor-parallel over heads (4 head-groups).
Core c handles batch (c // 4), heads [4*(c%4) .. 4*(c%4)+4) of both the MLA path and the
pattern path. The low-rank a-projections (q_a, kv_a lora) and the gate are replicated
within a batch group. Each core emits a partial (already gate-weighted) output
[S, HID]; the host sums the 4 partials per batch. No device collectives.

All matmuls run in bf16 (f32 PSUM accumulation); softmax/rmsnorm statistics in f32.
Weight preprocessing (transposes, ln-weight folding, scale folding, rope tables) is
done on host in numpy and shipped per-core via in_maps.
"""

import numpy as np
import ml_dtypes

# ---- model config (hardcoded from the problem spec) ----
B, S, HID = 2, 1024, 2048
H = 16
D_NOPE, D_ROPE, D_V = 128, 64, 128
D_Q = D_NOPE + D_ROPE            # 192
QR, KVR = 1536, 512
PH, PD = 16, 128
THETA, EPS = 10000.0, 1e-6
NCORES = 8
HPC = 4                          # heads per core
TB = S // 128                    # 8 token blocks
KT_HID = HID // 128              # 16
KT_QR = QR // 128                # 12
KT_KVR = KVR // 128              # 4

BF16 = ml_dtypes.bfloat16

# knobs for test harness
TRACE = False
RUN_KWARGS = {}
LAST_RESULT = None

_graph_cache = {}


def _build_graph():
    import concourse.bass as bass
    import concourse.mybir as mybir
    import concourse.tile as tile
    from concourse import bacc

    BF = mybir.dt.bfloat16
    F32 = mybir.dt.float32
    Exp = mybir.ActivationFunctionType.Exp
    Square = mybir.ActivationFunctionType.Square
    Sqrt = mybir.ActivationFunctionType.Sqrt
    MULT = mybir.AluOpType.mult
    ADD = mybir.AluOpType.add
    X = mybir.AxisListType.X
    ts = bass.ts

    nc = bacc.Bacc("TRN2", target_bir_lowering=False, debug=False,
                   num_devices=NCORES)

    def din(name, shape, dt=BF):
        return nc.declare_dram_parameter(name, list(shape), dt, isOutput=False)

    xT_d = din("xT", [HID, S])
    qa_d = din("qa_wT", [HID, QR])
    qbn_d = din("qbn_wT", [QR, HPC * D_NOPE])
    qbp_d = din("qbp_wT", [QR, HPC * D_ROPE])
    kvl_d = din("kvl_wT", [HID, KVR])
    kvp_d = din("kvp_wT", [HID, HPC * D_ROPE])
    kbn_d = din("kbn_wT", [KVR, HPC * D_NOPE])
    kbv_d = din("kbv_wT", [KVR, HPC * D_V])
    ow_d = din("o_wT", [HPC * D_V, HID])
    spq_d = din("spq_wT", [HID, HPC * PD])
    spk_d = din("spk_wT", [HID, HPC * PD])
    spv_d = din("spv_wT", [HID, HPC * PD])
    spo_d = din("spo_wT", [HPC * PD, HID])
    gw_d = din("gate_wT", [HID, 2])
    gb_d = din("gate_bias", [128, 2], F32)
    cos_d = din("cos2T", [128, S])
    sin_d = din("sin2T", [128, S])
    out_d = nc.declare_dram_parameter("out", [S, HID], F32, isOutput=True)

    def r3(dram, kt):
        # [kt*128, N] dram tensor viewed as [128, kt, N] for SBUF tiling
        return dram.ap().rearrange("(k p) n -> p k n", p=128, k=kt)

    with tile.TileContext(nc) as tc:
        with (
            tc.tile_pool(name="const", bufs=1) as constp,
            tc.tile_pool(name="small", bufs=4) as small,
            tc.tile_pool(name="pp", bufs=3, space="PSUM") as pp,
            tc.tile_pool(name="pt", bufs=3, space="PSUM") as pt,
            tc.tile_pool(name="psum1", bufs=1, space="PSUM") as psum1,
            tc.tile_pool(name="ain_mla", bufs=1) as ain,
        ):
            eps_t = constp.tile([128, 1], F32, tag="eps")
            nc.vector.memset(eps_t[:], EPS)
            ones_col = constp.tile([128, 1], BF, tag="ones_col")
            nc.vector.memset(ones_col[:], 1.0)
            ones_row = constp.tile([1, 128], BF, tag="ones_row")
            nc.vector.memset(ones_row[:], 1.0)
            cosT = constp.tile([128, S], BF, tag="cos")
            sinT = constp.tile([128, S], BF, tag="sin")
            nc.sync.dma_start(out=cosT[:], in_=cos_d.ap())
            nc.sync.dma_start(out=sinT[:], in_=sin_d.ap())
            gbias = constp.tile([128, 2], F32, tag="gb")
            nc.sync.dma_start(out=gbias[:], in_=gb_d.ap())
            g0_s = constp.tile([128, TB], F32, tag="g0")
            g1_s = constp.tile([128, TB], F32, tag="g1")

            # MLA attention inputs (feature-major unless noted)
            qnopeT = ain.tile([128, HPC, S], BF, tag="qnopeT")
            qpeT = ain.tile([128, 2, S], BF, tag="qpeT")
            knopeT = ain.tile([128, HPC, S], BF, tag="knopeT")
            kpeT = ain.tile([128, 2, S], BF, tag="kpeT")
            v_s = ain.tile([128, TB, HPC * D_V], BF, tag="v")      # token-major

            def dma_k(tile_, dram_r3, kt, nslice=None):
                for k in range(kt):
                    src = dram_r3[:, k, :] if nslice is None else dram_r3[:, k, nslice]
                    nc.sync.dma_start(out=tile_[:, k, :], in_=src)

            def rope_from_psum(ps, dst, nck, work):
                """Apply rope to a [128, 512] psum chunk holding 2 stacked
                64-dim pe heads; write bf16 to dst ([128,512] slice)."""
                rot = work.tile([128, 512], F32, tag="rot")
                nc.vector.tensor_scalar_mul(rot[0:32, :], ps[32:64, :], -1.0)
                nc.vector.tensor_copy(rot[32:64, :], ps[0:32, :])
                nc.vector.tensor_scalar_mul(rot[64:96, :], ps[96:128, :], -1.0)
                nc.vector.tensor_copy(rot[96:128, :], ps[64:96, :])
                t1 = work.tile([128, 512], F32, tag="t1")
                nc.vector.tensor_mul(t1[:], ps[:], cosT[:, ts(nck, 512)])
                nc.vector.tensor_mul(rot[:], rot[:], sinT[:, ts(nck, 512)])
                nc.vector.tensor_add(dst, t1[:], rot[:])

            def rms_scale_rows(ssq_ps, width, scale_k, dstT, mtiles, wk):
                """Given ssq row psum [1, width], scale dstT tiles in place by
                1/rms broadcast across partitions (via PE outer product)."""
                rms_row = small.tile([1, width], F32, tag="rmsrow")
                nc.scalar.activation(rms_row[:], ssq_ps[0:1, 0:width], Sqrt,
                                     bias=eps_t[0:1, :], scale=scale_k)
                inv_row = small.tile([1, width], F32, tag="invrow1")
                nc.vector.reciprocal(inv_row[:], rms_row[:])
                invb_row = small.tile([1, width], BF, tag="invrowb1")
                nc.vector.tensor_copy(invb_row[:], inv_row[:])
                for nck in range(width // 512):
                    bcp = pt.tile([128, 512], F32, tag="pt")
                    nc.tensor.matmul(bcp[:], lhsT=ones_row[:],
                                     rhs=invb_row[0:1, ts(nck, 512)],
                                     start=True, stop=True)
                    bcs = wk.tile([128, 512], BF, tag="bcs")
                    nc.any.tensor_copy(bcs[:], bcp[:])
                    for m in range(mtiles):
                        nc.vector.tensor_mul(dstT[:, m, ts(nck, 512)],
                                             dstT[:, m, ts(nck, 512)], bcs[:])

            with tc.tile_pool(name="xp", bufs=1) as xp:
                xT = xp.tile([128, KT_HID, S], BF, tag="xT")
                dma_k(xT, r3(xT_d, KT_HID), KT_HID)

                with tc.tile_pool(name="wrope", bufs=2) as wrope:
                    # ---------- Stage 1+2 ----------
                    with tc.tile_pool(name="q2", bufs=1) as q2:
                        qmidT = q2.tile([128, KT_QR, S], BF, tag="qmidT")
                        kvnT = q2.tile([128, KT_KVR, S], BF, tag="kvnT")

                        with (
                            tc.tile_pool(name="w1", bufs=1) as w1,
                            tc.tile_pool(name="wk1", bufs=3) as wk1,
                        ):
                            # q_a: feature-major [QR, S], rmsnorm stats via
                            # ACT square + PE ones-reduction
                            ssq_q = psum1.tile([1, 1024], F32, tag="sm")
                            for ck in range(3):
                                wt = w1.tile([128, KT_HID, 512], BF, tag="w")
                                dma_k(wt, r3(qa_d, KT_HID), KT_HID, ts(ck, 512))
                                for mm4 in range(4):
                                    m = ck * 4 + mm4
                                    for nck in range(2):
                                        ps = pp.tile([128, 512], F32, tag="pp")
                                        for k in range(KT_HID):
                                            nc.tensor.matmul(
                                                ps[:], lhsT=wt[:, k, ts(mm4, 128)],
                                                rhs=xT[:, k, ts(nck, 512)],
                                                start=(k == 0), stop=(k == KT_HID - 1))
                                        nc.any.tensor_copy(qmidT[:, m, ts(nck, 512)], ps[:])
                                        sq = wk1.tile([128, 512], BF, tag="sq")
                                        nc.scalar.activation(sq[:], ps[:], Square)
                                        nc.tensor.matmul(
                                            ssq_q[0:1, ts(nck, 512)], lhsT=ones_col[:],
                                            rhs=sq[:], start=(m == 0),
                                            stop=(m == KT_QR - 1))
                            rms_scale_rows(ssq_q, 1024, 1.0 / QR, qmidT, KT_QR, wk1)

                            # kv_a lora part: feature-major [KVR, S]
                            ssq_k = psum1.tile([1, 1024], F32, tag="sm")
                            wt_kl = w1.tile([128, KT_HID, 512], BF, tag="w")
                            dma_k(wt_kl, r3(kvl_d, KT_HID), KT_HID)
                            for m in range(KT_KVR):
                                for nck in range(2):
                                    ps = pp.tile([128, 512], F32, tag="pp")
                                    for k in range(KT_HID):
                                        nc.tensor.matmul(
                                            ps[:], lhsT=wt_kl[:, k, ts(m, 128)],
                                            rhs=xT[:, k, ts(nck, 512)],
                                            start=(k == 0), stop=(k == KT_HID - 1))
                                    nc.any.tensor_copy(kvnT[:, m, ts(nck, 512)], ps[:])
                                    sq = wk1.tile([128, 512], BF, tag="sq")
                                    nc.scalar.activation(sq[:], ps[:], Square)
                                    nc.tensor.matmul(
                                        ssq_k[0:1, ts(nck, 512)], lhsT=ones_col[:],
                                        rhs=sq[:], start=(m == 0),
                                        stop=(m == KT_KVR - 1))
                            rms_scale_rows(ssq_k, 1024, 1.0 / KVR, kvnT, KT_KVR, wk1)

                            # kv_a pe part: feature-major (2 heads per M-tile) + rope
                            wt_kp = w1.tile([128, KT_HID, HPC * D_ROPE], BF, tag="wkp")
                            dma_k(wt_kp, r3(kvp_d, KT_HID), KT_HID)
                            for m in range(2):
                                for nck in range(2):
                                    ps = pt.tile([128, 512], F32, tag="pt")
                                    for k in range(KT_HID):
                                        nc.tensor.matmul(
                                            ps[:], lhsT=wt_kp[:, k, ts(m, 128)],
                                            rhs=xT[:, k, ts(nck, 512)],
                                            start=(k == 0), stop=(k == KT_HID - 1))
                                    rope_from_psum(ps, kpeT[:, m, ts(nck, 512)], nck, wrope)

                        # ---------- Stage 2: b-projections ----------
                        with tc.tile_pool(name="w2", bufs=1) as w2:
                            wqbn = w2.tile([128, KT_QR, HPC * D_NOPE], BF, tag="wqbn")
                            dma_k(wqbn, r3(qbn_d, KT_QR), KT_QR)
                            wqbp = w2.tile([128, KT_QR, HPC * D_ROPE], BF, tag="wqbp")
                            dma_k(wqbp, r3(qbp_d, KT_QR), KT_QR)
                            wkbn = w2.tile([128, KT_KVR, HPC * D_NOPE], BF, tag="wkbn")
                            dma_k(wkbn, r3(kbn_d, KT_KVR), KT_KVR)
                            wkbv = w2.tile([128, KT_KVR, HPC * D_V], BF, tag="wkbv")
                            dma_k(wkbv, r3(kbv_d, KT_KVR), KT_KVR)

                            for h in range(HPC):
                                for nck in range(2):
                                    ps = pt.tile([128, 512], F32, tag="pt")
                                    for k in range(KT_QR):
                                        nc.tensor.matmul(
                                            ps[:], lhsT=wqbn[:, k, ts(h, 128)],
                                            rhs=qmidT[:, k, ts(nck, 512)],
                                            start=(k == 0), stop=(k == KT_QR - 1))
                                    nc.any.tensor_copy(qnopeT[:, h, ts(nck, 512)], ps[:])
                            for m in range(2):
                                for nck in range(2):
                                    ps = pt.tile([128, 512], F32, tag="pt")
                                    for k in range(KT_QR):
                                        nc.tensor.matmul(
                                            ps[:], lhsT=wqbp[:, k, ts(m, 128)],
                                            rhs=qmidT[:, k, ts(nck, 512)],
                                            start=(k == 0), stop=(k == KT_QR - 1))
                                    rope_from_psum(ps, qpeT[:, m, ts(nck, 512)], nck, wrope)
                            for h in range(HPC):
                                for nck in range(2):
                                    ps = pt.tile([128, 512], F32, tag="pt")
                                    for k in range(KT_KVR):
                                        nc.tensor.matmul(
                                            ps[:], lhsT=wkbn[:, k, ts(h, 128)],
                                            rhs=kvnT[:, k, ts(nck, 512)],
                                            start=(k == 0), stop=(k == KT_KVR - 1))
                                    nc.any.tensor_copy(knopeT[:, h, ts(nck, 512)], ps[:])
                            for tb in range(TB):
                                ps = pt.tile([128, 512], F32, tag="pt")
                                for k in range(KT_KVR):
                                    nc.tensor.matmul(
                                        ps[:], lhsT=kvnT[:, k, ts(tb, 128)],
                                        rhs=wkbv[:, k, :],
                                        start=(k == 0), stop=(k == KT_KVR - 1))
                                nc.any.tensor_copy(v_s[:, tb, :], ps[:])

                # ---------- attention helper ----------
                # k-major formulation: scoresT[k,q] on PE, unnormalized exp
                # (|score| <= ||q||*||k||/sqrt(D) stays well inside f32 exp
                # range for this model), v-stationary ctx matmuls at N=512,
                # denominators via DVE tree-sum + one PE ones-reduction,
                # normalization via an outer-product broadcast.
                def attention(h, qh, qnT, knT, vv, voff, ctxT, is_main, awk):
                    probsT = awk.tile([128, TB, 512], BF, tag="probsT")
                    for kb in range(TB):
                        ps = pp.tile([128, 512], F32, tag="pp")
                        nc.tensor.matmul(ps[:], lhsT=knT[:, h, ts(kb, 128)],
                                         rhs=qnT[:, h, ts(qh, 512)],
                                         start=True, stop=not is_main)
                        if is_main:
                            pb = (h % 2) * 64
                            nc.tensor.matmul(
                                ps[:],
                                lhsT=kpeT[pb:pb + 64, h // 2, ts(kb, 128)],
                                rhs=qpeT[pb:pb + 64, h // 2, ts(qh, 512)],
                                start=False, stop=True)
                        nc.scalar.activation(probsT[:, kb, :], ps[:], Exp)
                    # denominator: tree-sum over kb on DVE, then one PE
                    # ones-reduction over k-partitions
                    tr = [awk.tile([128, 512], BF, tag=f"tr{i}", name=f"tr{i}")
                          for i in range(4)]
                    for i in range(4):
                        nc.vector.tensor_add(tr[i][:], probsT[:, 2 * i, :],
                                             probsT[:, 2 * i + 1, :])
                    nc.vector.tensor_add(tr[0][:], tr[0][:], tr[1][:])
                    nc.vector.tensor_add(tr[2][:], tr[2][:], tr[3][:])
                    nc.vector.tensor_add(tr[0][:], tr[0][:], tr[2][:])
                    ct = pt.tile([128, 512], F32, tag="pt")
                    for kb in range(TB):
                        nc.tensor.matmul(ct[:], lhsT=vv[:, kb, voff:voff + 128],
                                         rhs=probsT[:, kb, :],
                                         start=(kb == 0), stop=(kb == TB - 1))
                    sm = psum1.tile([1, 1024], F32, tag="sm")
                    nc.tensor.matmul(sm[0:1, 0:512], lhsT=ones_col[:], rhs=tr[0][:],
                                     start=True, stop=True)
                    inv = small.tile([1, 512], F32, tag="invrow")
                    nc.vector.reciprocal(inv[:], sm[0:1, 0:512])
                    invb = small.tile([1, 512], BF, tag="invrowb")
                    nc.vector.tensor_copy(invb[:], inv[:])
                    bc = pt.tile([128, 512], F32, tag="pt")
                    nc.tensor.matmul(bc[:], lhsT=ones_row[:], rhs=invb[:],
                                     start=True, stop=True)
                    bcs = awk.tile([128, 512], F32, tag="bcs")
                    nc.any.tensor_copy(bcs[:], bc[:])
                    nc.vector.tensor_mul(ctxT[:, h, ts(qh, 512)], ct[:], bcs[:])

                with tc.tile_pool(name="ctxp", bufs=1) as ctxp:
                    ctxT_m = ctxp.tile([128, HPC, S], BF, tag="ctxm")
                    ctxT_p = ctxp.tile([128, HPC, S], BF, tag="ctxp")

                    # ---------- Stage 4a: MLA attention ----------
                    with tc.tile_pool(name="awk", bufs=2) as awk:
                        for h in range(HPC):
                            for qh in range(2):
                                attention(h, qh, qnopeT, knopeT,
                                          v_s, h * D_V, ctxT_m, True, awk)

                    # ---------- Stage 3: pattern projections + gate ----------
                    with (
                        tc.tile_pool(name="ain_pat", bufs=1) as ainp,
                        tc.tile_pool(name="w3", bufs=1) as w3,
                    ):
                        pqT = ainp.tile([128, HPC, S], BF, tag="pqT")
                        pkT = ainp.tile([128, HPC, S], BF, tag="pkT")
                        pv_s = ainp.tile([128, TB, HPC * PD], BF, tag="pv")

                        wspq = w3.tile([128, KT_HID, HPC * PD], BF, tag="w")
                        dma_k(wspq, r3(spq_d, KT_HID), KT_HID)
                        for m in range(HPC):
                            for nck in range(2):
                                ps = pt.tile([128, 512], F32, tag="pt")
                                for k in range(KT_HID):
                                    nc.tensor.matmul(
                                        ps[:], lhsT=wspq[:, k, ts(m, 128)],
                                        rhs=xT[:, k, ts(nck, 512)],
                                        start=(k == 0), stop=(k == KT_HID - 1))
                                nc.any.tensor_copy(pqT[:, m, ts(nck, 512)], ps[:])
                        wspk = w3.tile([128, KT_HID, HPC * PD], BF, tag="w")
                        dma_k(wspk, r3(spk_d, KT_HID), KT_HID)
                        for m in range(HPC):
                            for nck in range(2):
                                ps = pt.tile([128, 512], F32, tag="pt")
                                for k in range(KT_HID):
                                    nc.tensor.matmul(
                                        ps[:], lhsT=wspk[:, k, ts(m, 128)],
                                        rhs=xT[:, k, ts(nck, 512)],
                                        start=(k == 0), stop=(k == KT_HID - 1))
                                nc.any.tensor_copy(pkT[:, m, ts(nck, 512)], ps[:])
                        wspv = w3.tile([128, KT_HID, HPC * PD], BF, tag="w")
                        dma_k(wspv, r3(spv_d, KT_HID), KT_HID)
                        for tb in range(TB):
                            ps = pt.tile([128, 512], F32, tag="pt")
                            for k in range(KT_HID):
                                nc.tensor.matmul(
                                    ps[:], lhsT=xT[:, k, ts(tb, 128)],
                                    rhs=wspv[:, k, :],
                                    start=(k == 0), stop=(k == KT_HID - 1))
                            nc.any.tensor_copy(pv_s[:, tb, :], ps[:])

                        # gate
                        gwt = w3.tile([128, KT_HID, 2], BF, tag="gw")
                        dma_k(gwt, r3(gw_d, KT_HID), KT_HID)
                        for tb in range(TB):
                            psg = pp.tile([128, 2], F32, tag="pp")
                            for k in range(KT_HID):
                                nc.tensor.matmul(psg[:], lhsT=xT[:, k, ts(tb, 128)],
                                                 rhs=gwt[:, k, :],
                                                 start=(k == 0), stop=(k == KT_HID - 1))
                            glog = small.tile([128, 2], F32, tag="glog")
                            nc.vector.tensor_add(glog[:], psg[:], gbias[:])
                            gm = small.tile([128, 1], F32, tag="gm")
                            nc.vector.reduce_max(gm[:], glog[:], axis=X)
                            nc.vector.tensor_scalar_mul(gm[:], gm[:], -1.0)
                            gexp = small.tile([128, 2], F32, tag="gexp")
                            gsum = small.tile([128, 1], F32, tag="gsum")
                            nc.scalar.activation(gexp[:], glog[:], Exp, bias=gm[:],
                                                 accum_out=gsum[:])
                            ginv = small.tile([128, 1], F32, tag="ginv")
                            nc.vector.reciprocal(ginv[:], gsum[:])
                            nc.vector.tensor_scalar_mul(g0_s[:, tb:tb + 1],
                                                        gexp[:, 0:1], ginv[:])
                            nc.vector.tensor_scalar_mul(g1_s[:, tb:tb + 1],
                                                        gexp[:, 1:2], ginv[:])

                        # ---------- Stage 4b: pattern attention ----------
                        with tc.tile_pool(name="awk2", bufs=2) as awk2:
                            for h in range(HPC):
                                for qh in range(2):
                                    attention(h, qh, pqT, pkT,
                                              pv_s, h * PD, ctxT_p, False, awk2)

                    # ---------- Stage 5: output projections + gate combine ----------
                    with (
                        tc.tile_pool(name="w5", bufs=1) as w5,
                        tc.tile_pool(name="ow", bufs=2) as ow,
                    ):
                        wo = w5.tile([128, KT_KVR, HID], BF, tag="wo")
                        dma_k(wo, r3(ow_d, KT_KVR), KT_KVR)
                        wspo = w5.tile([128, KT_KVR, HID], BF, tag="wspo")
                        dma_k(wspo, r3(spo_d, KT_KVR), KT_KVR)
                        for tb in range(TB):
                            osb = ow.tile([128, HID], F32, tag="osb")
                            for ck in range(4):
                                pm = pp.tile([128, 512], F32, tag="pp")
                                for k in range(KT_KVR):
                                    nc.tensor.matmul(
                                        pm[:], lhsT=ctxT_m[:, k, ts(tb, 128)],
                                        rhs=wo[:, k, ts(ck, 512)],
                                        start=(k == 0), stop=(k == KT_KVR - 1))
                                pq2 = pp.tile([128, 512], F32, tag="pp")
                                for k in range(KT_KVR):
                                    nc.tensor.matmul(
                                        pq2[:], lhsT=ctxT_p[:, k, ts(tb, 128)],
                                        rhs=wspo[:, k, ts(ck, 512)],
                                        start=(k == 0), stop=(k == KT_KVR - 1))
                                tmp = ow.tile([128, 512], F32, tag="tmp")
                                nc.vector.tensor_scalar_mul(tmp[:], pq2[:],
                                                            g1_s[:, tb:tb + 1])
                                nc.vector.scalar_tensor_tensor(
                                    osb[:, ts(ck, 512)], in0=pm[:],
                                    scalar=g0_s[:, tb:tb + 1],
                                    in1=tmp[:], op0=MULT, op1=ADD)
                            nc.sync.dma_start(out=out_d[ts(tb, 128), :], in_=osb[:])

    nc.compile()
    return nc


def _rope_tables():
    inv_freq = 1.0 / (THETA ** (np.arange(0, D_ROPE, 2, dtype=np.float32) / D_ROPE))
    t = np.arange(S, dtype=np.float32)
    freqs = np.outer(t, inv_freq)                       # [S, 32]
    emb = np.concatenate([freqs, freqs], -1)            # [S, 64]
    cosT = np.cos(emb).T.astype(np.float32)             # [64, S]
    sinT = np.sin(emb).T.astype(np.float32)
    cos2T = np.ascontiguousarray(np.concatenate([cosT, cosT], 0))   # [128, S]
    sin2T = np.ascontiguousarray(np.concatenate([sinT, sinT], 0))
    return cos2T.astype(BF16), sin2T.astype(BF16)


def _prep_in_maps(hidden_states, q_a_w, q_a_ln_w, q_b_w, kv_a_w, kv_a_ln_w,
                  kv_b_w, o_w, sp_q_w, sp_k_w, sp_v_w, sp_o_w, gate_w, gate_b):
    def bf(x):
        return np.ascontiguousarray(x).astype(BF16)

    cos2T, sin2T = _rope_tables()
    qa_wT = bf(q_a_w.T)                                   # [HID, QR]
    kvl_wT = bf(kv_a_w[:KVR].T)                           # [HID, KVR]
    kv_a_pe = kv_a_w[KVR:].reshape(H, D_ROPE, HID)        # [H, 64, HID]

    qb = (q_b_w * q_a_ln_w[None, :]).reshape(H, D_Q, QR) * (D_Q ** -0.5)
    qb_nope = qb[:, :D_NOPE]                              # [H,128,QR]
    qb_pe = qb[:, D_NOPE:]                                # [H,64,QR]
    kvb = (kv_b_w * kv_a_ln_w[None, :]).reshape(H, D_NOPE + D_V, KVR)
    kb_nope = kvb[:, :D_NOPE]                             # [H,128,KVR]
    kb_v = kvb[:, D_NOPE:]                                # [H,128,KVR]
    o_wh = o_w.reshape(HID, H, D_V)                       # [HID,H,128]
    spq = (sp_q_w * (PD ** -0.5)).reshape(PH, PD, HID)
    spk = sp_k_w.reshape(PH, PD, HID)
    spv = sp_v_w.reshape(PH, PD, HID)
    spo = sp_o_w.reshape(HID, PH, PD)
    gate_wT = bf(gate_w.T)                                # [HID, 2]
    gate_bias = np.ascontiguousarray(
        np.broadcast_to(gate_b[None, :], (128, 2))).astype(np.float32)

    in_maps = []
    for c in range(NCORES):
        b, g = c // 4, c % 4
        hs = slice(4 * g, 4 * g + 4)
        m = {
            "xT": bf(hidden_states[b].T),
            "qa_wT": qa_wT,
            "qbn_wT": bf(qb_nope[hs].reshape(HPC * D_NOPE, QR).T),
            "qbp_wT": bf(qb_pe[hs].reshape(HPC * D_ROPE, QR).T),
            "kvl_wT": kvl_wT,
            "kvp_wT": bf(kv_a_pe[hs].reshape(HPC * D_ROPE, HID).T),
            "kbn_wT": bf(kb_nope[hs].reshape(HPC * D_NOPE, KVR).T),
            "kbv_wT": bf(kb_v[hs].reshape(HPC * D_V, KVR).T),
            "o_wT": bf(o_wh[:, hs].reshape(HID, HPC * D_V).T),
            "spq_wT": bf(spq[hs].reshape(HPC * PD, HID).T),
            "spk_wT": bf(spk[hs].reshape(HPC * PD, HID).T),
            "spv_wT": bf(spv[hs].reshape(HPC * PD, HID).T),
            "spo_wT": bf(spo[:, hs].reshape(HID, HPC * PD).T),
            "gate_wT": gate_wT,
            "gate_bias": gate_bias,
            "cos2T": cos2T,
            "sin2T": sin2T,
        }
        in_maps.append(m)
    return in_maps


def kernel(**inputs):
    global LAST_RESULT
    from concourse.bass_utils import run_bass_kernel_spmd

    inputs = {k: np.asarray(v) for k, v in inputs.items()}
    if "nc" not in _graph_cache:
        _graph_cache["nc"] = _build_graph()
    nc = _graph_cache["nc"]

    in_maps = _prep_in_maps(**inputs)
    res = run_bass_kernel_spmd(nc, in_maps, core_ids=list(range(NCORES)),
                               trace=TRACE, **RUN_KWARGS)
    LAST_RESULT = res
    out = np.zeros((B, S, HID), np.float32)
    for c in range(NCORES):
        out[c // 4] += res.results[c]["out"]
    return out


# revision 17
# speedup vs baseline: 1.0286x; 1.0286x over previous
"""ARCAttention (MLA + pattern-attention + gate) distributed Bass kernel for 8 TRN2 NeuronCores.

Sharding: data-parallel over batch (B=2) x tensor-parallel over heads (4 head-groups).
Core c handles batch (c // 4), heads [4*(c%4) .. 4*(c%4)+4) of both the MLA path and the
pattern path. The low-rank a-projections (q_a, kv_a lora) and the gate are replicated
within a batch group. Each core emits a partial (already gate-weighted) output
[S, HID]; the host sums the 4 partials per batch. No device collectives.

All matmuls run in bf16 (f32 PSUM accumulation); softmax/rmsnorm statistics in f32.
Weight preprocessing (transposes, ln-weight folding, scale folding, rope tables) is
done on host in numpy and shipped per-core via in_maps.
"""

import numpy as np
import ml_dtypes

# ---- model config (hardcoded from the problem spec) ----
B, S, HID = 2, 1024, 2048
H = 16
D_NOPE, D_ROPE, D_V = 128, 64, 128
D_Q = D_NOPE + D_ROPE            # 192
QR, KVR = 1536, 512
PH, PD = 16, 128
THETA, EPS = 10000.0, 1e-6
NCORES = 8
HPC = 4                          # heads per core
TB = S // 128                    # 8 token blocks
KT_HID = HID // 128              # 16
KT_QR = QR // 128                # 12
KT_KVR = KVR // 128              # 4

BF16 = ml_dtypes.bfloat16

# knobs for test harness
TRACE = False
RUN_KWARGS = {}
LAST_RESULT = None

_graph_cache = {}


def _build_graph():
    import concourse.bass as bass
    import concourse.mybir as mybir
    import concourse.tile as tile
    from concourse import bacc, bass_isa

    BF = mybir.dt.bfloat16
    F32 = mybir.dt.float32
    Exp = mybir.ActivationFunctionType.Exp
    Square = mybir.ActivationFunctionType.Square
    Sqrt = mybir.ActivationFunctionType.Sqrt
    MULT = mybir.AluOpType.mult
    ADD = mybir.AluOpType.add
    X = mybir.AxisListType.X
    ts = bass.ts

    nc = bacc.Bacc("TRN2", target_bir_lowering=False, debug=False,
                   num_devices=NCORES)

    def din(name, shape, dt=BF):
        return nc.declare_dram_parameter(name, list(shape), dt, isOutput=False)

    xT_d = din("xT", [HID, S])
    qa_d = din("qa_wT", [HID, QR])
    qbn_d = din("qbn_wT", [QR, HPC * D_NOPE])
    qbp_d = din("qbp_wT", [QR, HPC * D_ROPE])
    kvl_d = din("kvl_wT", [HID, KVR])
    kvp_d = din("kvp_wT", [HID, HPC * D_ROPE])
    kbn_d = din("kbn_wT", [KVR, HPC * D_NOPE])
    kbv_d = din("kbv_wT", [KVR, HPC * D_V])
    ow_d = din("o_wT", [HPC * D_V, HID])
    spq_d = din("spq_wT", [HID, HPC * PD])
    spk_d = din("spk_wT", [HID, HPC * PD])
    spv_d = din("spv_wT", [HID, HPC * PD])
    spo_d = din("spo_wT", [HPC * PD, HID])
    gw_d = din("gate_wT", [HID, 2])
    gb_d = din("gate_bias", [128, 2], F32)
    cos_d = din("cos2T", [128, S])
    sin_d = din("sin2T", [128, S])
    out_d = nc.declare_dram_parameter("out", [S, HID], F32, isOutput=True)

    def r3(dram, kt):
        # [kt*128, N] dram tensor viewed as [128, kt, N] for SBUF tiling
        return dram.ap().rearrange("(k p) n -> p k n", p=128, k=kt)

    with tile.TileContext(nc) as tc:
        with (
            tc.tile_pool(name="const", bufs=1) as constp,
            tc.tile_pool(name="small", bufs=4) as small,
            tc.tile_pool(name="pp", bufs=4, space="PSUM") as pp,
            tc.tile_pool(name="pt", bufs=2, space="PSUM") as pt,
            tc.tile_pool(name="psum1", bufs=1, space="PSUM") as psum1,
            tc.tile_pool(name="ain_mla", bufs=1) as ain,
        ):
            eps_t = constp.tile([128, 1], F32, tag="eps")
            nc.vector.memset(eps_t[:], EPS)
            ones_col = constp.tile([128, 1], BF, tag="ones_col")
            nc.vector.memset(ones_col[:], 1.0)
            ones_row = constp.tile([1, 128], BF, tag="ones_row")
            nc.vector.memset(ones_row[:], 1.0)
            cosT = constp.tile([128, S], BF, tag="cos")
            sinT = constp.tile([128, S], BF, tag="sin")
            nc.sync.dma_start(out=cosT[:], in_=cos_d.ap())
            nc.sync.dma_start(out=sinT[:], in_=sin_d.ap())
            gbias = constp.tile([128, 2], F32, tag="gb")
            nc.sync.dma_start(out=gbias[:], in_=gb_d.ap())
            g0_s = constp.tile([128, TB], F32, tag="g0")
            g1_s = constp.tile([128, TB], F32, tag="g1")

            # MLA attention inputs (feature-major unless noted)
            qnopeT = ain.tile([128, HPC, S], BF, tag="qnopeT")
            qpeT = ain.tile([128, 2, S], BF, tag="qpeT")
            knopeT = ain.tile([128, HPC, S], BF, tag="knopeT")
            kpeT = ain.tile([128, 2, S], BF, tag="kpeT")
            v_s = ain.tile([128, TB, HPC * D_V], BF, tag="v")      # token-major

            def dma_k(tile_, dram_r3, kt, nslice=None):
                for k in range(kt):
                    src = dram_r3[:, k, :] if nslice is None else dram_r3[:, k, nslice]
                    nc.sync.dma_start(out=tile_[:, k, :], in_=src)

            def rope_from_psum(ps, dst, nck, work):
                """Apply rope to a [128, 512] psum chunk holding 2 stacked
                64-dim pe heads; write bf16 to dst ([128,512] slice)."""
                rot = work.tile([128, 512], F32, tag="rot")
                nc.vector.tensor_scalar_mul(rot[0:32, :], ps[32:64, :], -1.0)
                nc.vector.tensor_copy(rot[32:64, :], ps[0:32, :])
                nc.vector.tensor_scalar_mul(rot[64:96, :], ps[96:128, :], -1.0)
                nc.vector.tensor_copy(rot[96:128, :], ps[64:96, :])
                t1 = work.tile([128, 512], F32, tag="t1")
                nc.vector.tensor_mul(t1[:], ps[:], cosT[:, ts(nck, 512)])
                nc.vector.tensor_mul(rot[:], rot[:], sinT[:, ts(nck, 512)])
                nc.vector.tensor_add(dst, t1[:], rot[:])

            def rms_scale_rows(ssq_ps, width, scale_k, dstT, mtiles, wk):
                """Given ssq row psum [1, width], scale dstT tiles in place by
                1/rms broadcast across partitions (via PE outer product)."""
                rms_row = small.tile([1, width], F32, tag="rmsrow")
                nc.scalar.activation(rms_row[:], ssq_ps[0:1, 0:width], Sqrt,
                                     bias=eps_t[0:1, :], scale=scale_k)
                inv_row = small.tile([1, width], F32, tag="invrow1")
                nc.vector.reciprocal(inv_row[:], rms_row[:])
                invb_row = small.tile([1, width], BF, tag="invrowb1")
                nc.vector.tensor_copy(invb_row[:], inv_row[:])
                for nck in range(width // 512):
                    bcp = pt.tile([128, 512], F32, tag="pt")
                    nc.tensor.matmul(bcp[:], lhsT=ones_row[:],
                                     rhs=invb_row[0:1, ts(nck, 512)],
                                     start=True, stop=True)
                    bcs = wk.tile([128, 512], BF, tag="bcs")
                    nc.any.tensor_copy(bcs[:], bcp[:])
                    for m in range(mtiles):
                        nc.vector.tensor_mul(dstT[:, m, ts(nck, 512)],
                                             dstT[:, m, ts(nck, 512)], bcs[:])

            with tc.tile_pool(name="xp", bufs=1) as xp:
                xT = xp.tile([128, KT_HID, S], BF, tag="xT")
                dma_k(xT, r3(xT_d, KT_HID), KT_HID)

                with tc.tile_pool(name="wrope", bufs=2) as wrope:
                    # ---------- Stage 1+2 ----------
                    with tc.tile_pool(name="q2", bufs=1) as q2:
                        qmidT = q2.tile([128, KT_QR, S], BF, tag="qmidT")
                        kvnT = q2.tile([128, KT_KVR, S], BF, tag="kvnT")

                        with (
                            tc.tile_pool(name="w1", bufs=1) as w1,
                            tc.tile_pool(name="wk1", bufs=3) as wk1,
                        ):
                            # q_a: feature-major [QR, S], rmsnorm stats via
                            # ACT square + PE ones-reduction
                            ssq_q = psum1.tile([1, 1024], F32, tag="sm")
                            for ck in range(3):
                                wt = w1.tile([128, KT_HID, 512], BF, tag="w")
                                dma_k(wt, r3(qa_d, KT_HID), KT_HID, ts(ck, 512))
                                sqs = []
                                for mm4 in range(4):
                                    m = ck * 4 + mm4
                                    for nck in range(2):
                                        ps = pp.tile([128, 512], F32, tag="pp")
                                        for k in range(KT_HID):
                                            nc.tensor.matmul(
                                                ps[:], lhsT=wt[:, k, ts(mm4, 128)],
                                                rhs=xT[:, k, ts(nck, 512)],
                                                start=(k == 0), stop=(k == KT_HID - 1))
                                        nc.any.tensor_copy(qmidT[:, m, ts(nck, 512)], ps[:])
                                        sq = wk1.tile([128, 512], BF, tag="sq", bufs=8)
                                        nc.scalar.activation(sq[:], ps[:], Square)
                                        sqs.append((m, nck, sq))
                                for m, nck, sq in sqs:
                                    nc.tensor.matmul(
                                        ssq_q[0:1, ts(nck, 512)], lhsT=ones_col[:],
                                        rhs=sq[:], start=(m == 0),
                                        stop=(m == KT_QR - 1))
                            rms_scale_rows(ssq_q, 1024, 1.0 / QR, qmidT, KT_QR, wk1)

                            # kv_a lora part: feature-major [KVR, S]
                            ssq_k = psum1.tile([1, 1024], F32, tag="sm")
                            wt_kl = w1.tile([128, KT_HID, 512], BF, tag="w")
                            dma_k(wt_kl, r3(kvl_d, KT_HID), KT_HID)
                            sqs_k = []
                            for m in range(KT_KVR):
                                for nck in range(2):
                                    ps = pp.tile([128, 512], F32, tag="pp")
                                    for k in range(KT_HID):
                                        nc.tensor.matmul(
                                            ps[:], lhsT=wt_kl[:, k, ts(m, 128)],
                                            rhs=xT[:, k, ts(nck, 512)],
                                            start=(k == 0), stop=(k == KT_HID - 1))
                                    nc.any.tensor_copy(kvnT[:, m, ts(nck, 512)], ps[:])
                                    sq = wk1.tile([128, 512], BF, tag="sq", bufs=8)
                                    nc.scalar.activation(sq[:], ps[:], Square)
                                    sqs_k.append((m, nck, sq))
                            for m, nck, sq in sqs_k:
                                nc.tensor.matmul(
                                    ssq_k[0:1, ts(nck, 512)], lhsT=ones_col[:],
                                    rhs=sq[:], start=(m == 0),
                                    stop=(m == KT_KVR - 1))
                            rms_scale_rows(ssq_k, 1024, 1.0 / KVR, kvnT, KT_KVR, wk1)

                            # kv_a pe part: feature-major (2 heads per M-tile) + rope
                            wt_kp = w1.tile([128, KT_HID, HPC * D_ROPE], BF, tag="wkp")
                            dma_k(wt_kp, r3(kvp_d, KT_HID), KT_HID)
                            for m in range(2):
                                for nck in range(2):
                                    ps = pt.tile([128, 512], F32, tag="pt")
                                    for k in range(KT_HID):
                                        nc.tensor.matmul(
                                            ps[:], lhsT=wt_kp[:, k, ts(m, 128)],
                                            rhs=xT[:, k, ts(nck, 512)],
                                            start=(k == 0), stop=(k == KT_HID - 1))
                                    rope_from_psum(ps, kpeT[:, m, ts(nck, 512)], nck, wrope)

                        # ---------- Stage 2: b-projections ----------
                        with tc.tile_pool(name="w2", bufs=1) as w2:
                            wqbn = w2.tile([128, KT_QR, HPC * D_NOPE], BF, tag="wqbn")
                            dma_k(wqbn, r3(qbn_d, KT_QR), KT_QR)
                            wqbp = w2.tile([128, KT_QR, HPC * D_ROPE], BF, tag="wqbp")
                            dma_k(wqbp, r3(qbp_d, KT_QR), KT_QR)
                            wkbn = w2.tile([128, KT_KVR, HPC * D_NOPE], BF, tag="wkbn")
                            dma_k(wkbn, r3(kbn_d, KT_KVR), KT_KVR)
                            wkbv = w2.tile([128, KT_KVR, HPC * D_V], BF, tag="wkbv")
                            dma_k(wkbv, r3(kbv_d, KT_KVR), KT_KVR)

                            for h in range(HPC):
                                for nck in range(2):
                                    ps = pt.tile([128, 512], F32, tag="pt")
                                    for k in range(KT_QR):
                                        nc.tensor.matmul(
                                            ps[:], lhsT=wqbn[:, k, ts(h, 128)],
                                            rhs=qmidT[:, k, ts(nck, 512)],
                                            start=(k == 0), stop=(k == KT_QR - 1))
                                    nc.any.tensor_copy(qnopeT[:, h, ts(nck, 512)], ps[:])
                            for m in range(2):
                                for nck in range(2):
                                    ps = pt.tile([128, 512], F32, tag="pt")
                                    for k in range(KT_QR):
                                        nc.tensor.matmul(
                                            ps[:], lhsT=wqbp[:, k, ts(m, 128)],
                                            rhs=qmidT[:, k, ts(nck, 512)],
                                            start=(k == 0), stop=(k == KT_QR - 1))
                                    rope_from_psum(ps, qpeT[:, m, ts(nck, 512)], nck, wrope)
                            for h in range(HPC):
                                for nck in range(2):
                                    ps = pt.tile([128, 512], F32, tag="pt")
                                    for k in range(KT_KVR):
                                        nc.tensor.matmul(
                                            ps[:], lhsT=wkbn[:, k, ts(h, 128)],
                                            rhs=kvnT[:, k, ts(nck, 512)],
                                            start=(k == 0), stop=(k == KT_KVR - 1))
                                    nc.any.tensor_copy(knopeT[:, h, ts(nck, 512)], ps[:])
                            for tb in range(TB):
                                ps = pt.tile([128, 512], F32, tag="pt")
                                for k in range(KT_KVR):
                                    nc.tensor.matmul(
                                        ps[:], lhsT=kvnT[:, k, ts(tb, 128)],
                                        rhs=wkbv[:, k, :],
                                        start=(k == 0), stop=(k == KT_KVR - 1))
                                nc.any.tensor_copy(v_s[:, tb, :], ps[:])

                # ---------- attention helper ----------
                # k-major formulation: scoresT[k,q] on PE, unnormalized exp
                # (|score| <= ||q||*||k||/sqrt(D) stays well inside f32 exp
                # range for this model), v-stationary ctx matmuls at N=512,
                # denominators via DVE tree-sum + one PE ones-reduction,
                # normalization via an outer-product broadcast.
                def attention(h, qh, qnT, knT, vv, voff, ctxT, is_main, awk):
                    probsT = awk.tile([128, TB, 512], BF, tag="probsT")
                    for kb in range(TB):
                        ps = pp.tile([128, 512], F32, tag="pp")
                        nc.tensor.matmul(ps[:], lhsT=knT[:, h, ts(kb, 128)],
                                         rhs=qnT[:, h, ts(qh, 512)],
                                         start=True, stop=not is_main)
                        if is_main:
                            pb = (h % 2) * 64
                            nc.tensor.matmul(
                                ps[:],
                                lhsT=kpeT[pb:pb + 64, h // 2, ts(kb, 128)],
                                rhs=qpeT[pb:pb + 64, h // 2, ts(qh, 512)],
                                start=False, stop=True)
                        nc.scalar.activation(probsT[:, kb, :], ps[:], Exp)
                    # denominator: tree-sum over kb on DVE, then one PE
                    # ones-reduction over k-partitions
                    tr = [awk.tile([128, 512], BF, tag=f"tr{i}", name=f"tr{i}")
                          for i in range(4)]
                    for i in range(4):
                        nc.vector.tensor_add(tr[i][:], probsT[:, 2 * i, :],
                                             probsT[:, 2 * i + 1, :])
                    nc.vector.tensor_add(tr[0][:], tr[0][:], tr[1][:])
                    nc.vector.tensor_add(tr[2][:], tr[2][:], tr[3][:])
                    nc.vector.tensor_add(tr[0][:], tr[0][:], tr[2][:])
                    ct = pt.tile([128, 512], F32, tag="pt")
                    for kb in range(TB):
                        nc.tensor.matmul(ct[:], lhsT=vv[:, kb, voff:voff + 128],
                                         rhs=probsT[:, kb, :],
                                         start=(kb == 0), stop=(kb == TB - 1))
                    cts = awk.tile([128, 512], F32, tag="cts")
                    nc.any.tensor_copy(cts[:], ct[:])
                    ars = awk.tile([128, 512], F32, tag="ars")
                    nc.gpsimd.partition_all_reduce(ars[:], tr[0][:], 128,
                                                   bass_isa.ReduceOp.add)
                    inv = awk.tile([128, 512], F32, tag="inv")
                    nc.vector.reciprocal(inv[:], ars[:])
                    nc.vector.tensor_mul(ctxT[:, h, ts(qh, 512)], cts[:], inv[:])

                with tc.tile_pool(name="ctxp", bufs=1) as ctxp:
                    ctxT_m = ctxp.tile([128, HPC, S], BF, tag="ctxm")
                    ctxT_p = ctxp.tile([128, HPC, S], BF, tag="ctxp")

                    # ---------- Stage 4a: MLA attention ----------
                    with tc.tile_pool(name="awk", bufs=2) as awk:
                        for h in range(HPC):
                            for qh in range(2):
                                attention(h, qh, qnopeT, knopeT,
                                          v_s, h * D_V, ctxT_m, True, awk)

                    # ---------- Stage 3: pattern projections + gate ----------
                    with (
                        tc.tile_pool(name="ain_pat", bufs=1) as ainp,
                        tc.tile_pool(name="w3", bufs=1) as w3,
                    ):
                        pqT = ainp.tile([128, HPC, S], BF, tag="pqT")
                        pkT = ainp.tile([128, HPC, S], BF, tag="pkT")
                        pv_s = ainp.tile([128, TB, HPC * PD], BF, tag="pv")

                        wspq = w3.tile([128, KT_HID, HPC * PD], BF, tag="w")
                        dma_k(wspq, r3(spq_d, KT_HID), KT_HID)
                        for m in range(HPC):
                            for nck in range(2):
                                ps = pt.tile([128, 512], F32, tag="pt")
                                for k in range(KT_HID):
                                    nc.tensor.matmul(
                                        ps[:], lhsT=wspq[:, k, ts(m, 128)],
                                        rhs=xT[:, k, ts(nck, 512)],
                                        start=(k == 0), stop=(k == KT_HID - 1))
                                nc.any.tensor_copy(pqT[:, m, ts(nck, 512)], ps[:])
                        wspk = w3.tile([128, KT_HID, HPC * PD], BF, tag="w")
                        dma_k(wspk, r3(spk_d, KT_HID), KT_HID)
                        for m in range(HPC):
                            for nck in range(2):
                                ps = pt.tile([128, 512], F32, tag="pt")
                                for k in range(KT_HID):
                                    nc.tensor.matmul(
                                        ps[:], lhsT=wspk[:, k, ts(m, 128)],
                                        rhs=xT[:, k, ts(nck, 512)],
                                        start=(k == 0), stop=(k == KT_HID - 1))
                                nc.any.tensor_copy(pkT[:, m, ts(nck, 512)], ps[:])
                        wspv = w3.tile([128, KT_HID, HPC * PD], BF, tag="w")
                        dma_k(wspv, r3(spv_d, KT_HID), KT_HID)
                        for tb in range(TB):
                            ps = pt.tile([128, 512], F32, tag="pt")
                            for k in range(KT_HID):
                                nc.tensor.matmul(
                                    ps[:], lhsT=xT[:, k, ts(tb, 128)],
                                    rhs=wspv[:, k, :],
                                    start=(k == 0), stop=(k == KT_HID - 1))
                            nc.any.tensor_copy(pv_s[:, tb, :], ps[:])

                        # gate
                        gwt = w3.tile([128, KT_HID, 2], BF, tag="gw")
                        dma_k(gwt, r3(gw_d, KT_HID), KT_HID)
                        for tb in range(TB):
                            psg = pp.tile([128, 2], F32, tag="pp")
                            for k in range(KT_HID):
                                nc.tensor.matmul(psg[:], lhsT=xT[:, k, ts(tb, 128)],
                                                 rhs=gwt[:, k, :],
                                                 start=(k == 0), stop=(k == KT_HID - 1))
                            glog = small.tile([128, 2], F32, tag="glog")
                            nc.vector.tensor_add(glog[:], psg[:], gbias[:])
                            gm = small.tile([128, 1], F32, tag="gm")
                            nc.vector.reduce_max(gm[:], glog[:], axis=X)
                            nc.vector.tensor_scalar_mul(gm[:], gm[:], -1.0)
                            gexp = small.tile([128, 2], F32, tag="gexp")
                            gsum = small.tile([128, 1], F32, tag="gsum")
                            nc.scalar.activation(gexp[:], glog[:], Exp, bias=gm[:],
                                                 accum_out=gsum[:])
                            ginv = small.tile([128, 1], F32, tag="ginv")
                            nc.vector.reciprocal(ginv[:], gsum[:])
                            nc.vector.tensor_scalar_mul(g0_s[:, tb:tb + 1],
                                                        gexp[:, 0:1], ginv[:])
                            nc.vector.tensor_scalar_mul(g1_s[:, tb:tb + 1],
                                                        gexp[:, 1:2], ginv[:])

                        # ---------- Stage 4b: pattern attention ----------
                        with tc.tile_pool(name="awk2", bufs=2) as awk2:
                            for h in range(HPC):
                                for qh in range(2):
                                    attention(h, qh, pqT, pkT,
                                              pv_s, h * PD, ctxT_p, False, awk2)

                    # ---------- Stage 5: output projections + gate combine ----------
                    with (
                        tc.tile_pool(name="w5", bufs=1) as w5,
                        tc.tile_pool(name="ow", bufs=2) as ow,
                    ):
                        wo = w5.tile([128, KT_KVR, HID], BF, tag="wo")
                        dma_k(wo, r3(ow_d, KT_KVR), KT_KVR)
                        wspo = w5.tile([128, KT_KVR, HID], BF, tag="wspo")
                        dma_k(wspo, r3(spo_d, KT_KVR), KT_KVR)
                        for tb in range(TB):
                            osb = ow.tile([128, HID], F32, tag="osb")
                            for ck in range(4):
                                pm = pp.tile([128, 512], F32, tag="pp")
                                for k in range(KT_KVR):
                                    nc.tensor.matmul(
                                        pm[:], lhsT=ctxT_m[:, k, ts(tb, 128)],
                                        rhs=wo[:, k, ts(ck, 512)],
                                        start=(k == 0), stop=(k == KT_KVR - 1))
                                pq2 = pp.tile([128, 512], F32, tag="pp")
                                for k in range(KT_KVR):
                                    nc.tensor.matmul(
                                        pq2[:], lhsT=ctxT_p[:, k, ts(tb, 128)],
                                        rhs=wspo[:, k, ts(ck, 512)],
                                        start=(k == 0), stop=(k == KT_KVR - 1))
                                tmp = ow.tile([128, 512], F32, tag="tmp")
                                nc.vector.tensor_scalar_mul(tmp[:], pq2[:],
                                                            g1_s[:, tb:tb + 1])
                                nc.vector.scalar_tensor_tensor(
                                    osb[:, ts(ck, 512)], in0=pm[:],
                                    scalar=g0_s[:, tb:tb + 1],
                                    in1=tmp[:], op0=MULT, op1=ADD)
                            nc.sync.dma_start(out=out_d[ts(tb, 128), :], in_=osb[:])

    nc.compile()
    return nc


def _rope_tables():
    inv_freq = 1.0 / (THETA ** (np.arange(0, D_ROPE, 2, dtype=np.float32) / D_ROPE))
    t = np.arange(S, dtype=np.float32)
    freqs = np.outer(t, inv_freq)                       # [S, 32]
    emb = np.concatenate([freqs, freqs], -1)            # [S, 64]
    cosT = np.cos(emb).T.astype(np.float32)             # [64, S]
    sinT = np.sin(emb).T.astype(np.float32)
    cos2T = np.ascontiguousarray(np.concatenate([cosT, cosT], 0))   # [128, S]
    sin2T = np.ascontiguousarray(np.concatenate([sinT, sinT], 0))
    return cos2T.astype(BF16), sin2T.astype(BF16)


def _prep_in_maps(hidden_states, q_a_w, q_a_ln_w, q_b_w, kv_a_w, kv_a_ln_w,
                  kv_b_w, o_w, sp_q_w, sp_k_w, sp_v_w, sp_o_w, gate_w, gate_b):
    def bf(x):
        return np.ascontiguousarray(x).astype(BF16)

    cos2T, sin2T = _rope_tables()
    qa_wT = bf(q_a_w.T)                                   # [HID, QR]
    kvl_wT = bf(kv_a_w[:KVR].T)                           # [HID, KVR]
    kv_a_pe = kv_a_w[KVR:].reshape(H, D_ROPE, HID)        # [H, 64, HID]

    qb = (q_b_w * q_a_ln_w[None, :]).reshape(H, D_Q, QR) * (D_Q ** -0.5)
    qb_nope = qb[:, :D_NOPE]                              # [H,128,QR]
    qb_pe = qb[:, D_NOPE:]                                # [H,64,QR]
    kvb = (kv_b_w * kv_a_ln_w[None, :]).reshape(H, D_NOPE + D_V, KVR)
    kb_nope = kvb[:, :D_NOPE]                             # [H,128,KVR]
    kb_v = kvb[:, D_NOPE:]                                # [H,128,KVR]
    o_wh = o_w.reshape(HID, H, D_V)                       # [HID,H,128]
    spq = (sp_q_w * (PD ** -0.5)).reshape(PH, PD, HID)
    spk = sp_k_w.reshape(PH, PD, HID)
    spv = sp_v_w.reshape(PH, PD, HID)
    spo = sp_o_w.reshape(HID, PH, PD)
    gate_wT = bf(gate_w.T)                                # [HID, 2]
    gate_bias = np.ascontiguousarray(
        np.broadcast_to(gate_b[None, :], (128, 2))).astype(np.float32)

    in_maps = []
    for c in range(NCORES):
        b, g = c // 4, c % 4
        hs = slice(4 * g, 4 * g + 4)
        m = {
            "xT": bf(hidden_states[b].T),
            "qa_wT": qa_wT,
            "qbn_wT": bf(qb_nope[hs].reshape(HPC * D_NOPE, QR).T),
            "qbp_wT": bf(qb_pe[hs].reshape(HPC * D_ROPE, QR).T),
            "kvl_wT": kvl_wT,
            "kvp_wT": bf(kv_a_pe[hs].reshape(HPC * D_ROPE, HID).T),
            "kbn_wT": bf(kb_nope[hs].reshape(HPC * D_NOPE, KVR).T),
            "kbv_wT": bf(kb_v[hs].reshape(HPC * D_V, KVR).T),
            "o_wT": bf(o_wh[:, hs].reshape(HID, HPC * D_V).T),
            "spq_wT": bf(spq[hs].reshape(HPC * PD, HID).T),
            "spk_wT": bf(spk[hs].reshape(HPC * PD, HID).T),
            "spv_wT": bf(spv[hs].reshape(HPC * PD, HID).T),
            "spo_wT": bf(spo[:, hs].reshape(HID, HPC * PD).T),
            "gate_wT": gate_wT,
            "gate_bias": gate_bias,
            "cos2T": cos2T,
            "sin2T": sin2T,
        }
        in_maps.append(m)
    return in_maps


def kernel(**inputs):
    global LAST_RESULT
    from concourse.bass_utils import run_bass_kernel_spmd

    inputs = {k: np.asarray(v) for k, v in inputs.items()}
    if "nc" not in _graph_cache:
        _graph_cache["nc"] = _build_graph()
    nc = _graph_cache["nc"]

    in_maps = _prep_in_maps(**inputs)
    res = run_bass_kernel_spmd(nc, in_maps, core_ids=list(range(NCORES)),
                               trace=TRACE, **RUN_KWARGS)
    LAST_RESULT = res
    out = np.zeros((B, S, HID), np.float32)
    for c in range(NCORES):
        out[c // 4] += res.results[c]["out"]
    return out


# revision 24
# speedup vs baseline: 1.0633x; 1.0338x over previous
"""ARCAttention (MLA + pattern-attention + gate) distributed Bass kernel for 8 TRN2 NeuronCores.

Sharding: data-parallel over batch (B=2) x tensor-parallel over heads (4 head-groups).
Core c handles batch (c // 4), heads [4*(c%4) .. 4*(c%4)+4) of both the MLA path and the
pattern path. The low-rank a-projections (q_a, kv_a lora) and the gate are replicated
within a batch group. Each core emits a partial (already gate-weighted) output
[S, HID]; the host sums the 4 partials per batch. No device collectives.

All matmuls run in bf16 (f32 PSUM accumulation); softmax/rmsnorm statistics in f32.
Weight preprocessing (transposes, ln-weight folding, scale folding, rope tables) is
done on host in numpy and shipped per-core via in_maps.
"""

import numpy as np
import ml_dtypes

# ---- model config (hardcoded from the problem spec) ----
B, S, HID = 2, 1024, 2048
H = 16
D_NOPE, D_ROPE, D_V = 128, 64, 128
D_Q = D_NOPE + D_ROPE            # 192
QR, KVR = 1536, 512
PH, PD = 16, 128
THETA, EPS = 10000.0, 1e-6
NCORES = 8
HPC = 4                          # heads per core
TB = S // 128                    # 8 token blocks
KT_HID = HID // 128              # 16
KT_QR = QR // 128                # 12
KT_KVR = KVR // 128              # 4

BF16 = ml_dtypes.bfloat16

# knobs for test harness
TRACE = False
RUN_KWARGS = {}
LAST_RESULT = None

_graph_cache = {}


def _build_graph():
    from contextlib import ExitStack
    import concourse.bass as bass
    import concourse.mybir as mybir
    import concourse.tile as tile
    from concourse import bacc, bass_isa

    BF = mybir.dt.bfloat16
    F32 = mybir.dt.float32
    Exp = mybir.ActivationFunctionType.Exp
    Square = mybir.ActivationFunctionType.Square
    Sqrt = mybir.ActivationFunctionType.Sqrt
    MULT = mybir.AluOpType.mult
    ADD = mybir.AluOpType.add
    X = mybir.AxisListType.X
    ts = bass.ts

    nc = bacc.Bacc("TRN2", target_bir_lowering=False, debug=False,
                   num_devices=NCORES)

    def din(name, shape, dt=BF):
        return nc.declare_dram_parameter(name, list(shape), dt, isOutput=False)

    xT_d = din("xT", [HID, S])
    qa_d = din("qa_wT", [HID, QR])
    qbn_d = din("qbn_wT", [QR, HPC * D_NOPE])
    qbp_d = din("qbp_wT", [QR, HPC * D_ROPE])
    kvl_d = din("kvl_wT", [HID, KVR])
    kvp_d = din("kvp_wT", [HID, HPC * D_ROPE])
    kbn_d = din("kbn_wT", [KVR, HPC * D_NOPE])
    kbv_d = din("kbv_wT", [KVR, HPC * D_V])
    ow_d = din("o_wT", [HPC * D_V, HID])
    spq_d = din("spq_wT", [HID, HPC * PD])
    spk_d = din("spk_wT", [HID, HPC * PD])
    spv_d = din("spv_wT", [HID, HPC * PD])
    spo_d = din("spo_wT", [HPC * PD, HID])
    gw_d = din("gate_wT", [HID, 2])
    gb_d = din("gate_bias", [128, 2], F32)
    cos_d = din("cos2T", [128, S])
    sin_d = din("sin2T", [128, S])
    out_d = nc.declare_dram_parameter("out", [S, HID], F32, isOutput=True)

    def r3(dram, kt):
        # [kt*128, N] dram tensor viewed as [128, kt, N] for SBUF tiling
        return dram.ap().rearrange("(k p) n -> p k n", p=128, k=kt)

    es = ExitStack()
    with tile.TileContext(nc) as tc, es:
        constp = es.enter_context(tc.tile_pool(name="const", bufs=1))
        small = es.enter_context(tc.tile_pool(name="small", bufs=4))
        pp = es.enter_context(tc.tile_pool(name="pp", bufs=4, space="PSUM"))
        pt = es.enter_context(tc.tile_pool(name="pt", bufs=2, space="PSUM"))
        psum1 = es.enter_context(tc.tile_pool(name="psum1", bufs=1, space="PSUM"))
        wring = es.enter_context(tc.tile_pool(name="wring", bufs=3))
        xp = es.enter_context(tc.tile_pool(name="xp", bufs=1))
        ctxp = es.enter_context(tc.tile_pool(name="ctxp", bufs=1))

        eps_t = constp.tile([128, 1], F32, tag="eps")
        nc.vector.memset(eps_t[:], EPS)
        ones_col = constp.tile([128, 1], BF, tag="ones_col")
        nc.vector.memset(ones_col[:], 1.0)
        ones_row = constp.tile([1, 128], BF, tag="ones_row")
        nc.vector.memset(ones_row[:], 1.0)
        cosT = constp.tile([128, S], BF, tag="cos")
        sinT = constp.tile([128, S], BF, tag="sin")
        nc.sync.dma_start(out=cosT[:], in_=cos_d.ap())
        nc.sync.dma_start(out=sinT[:], in_=sin_d.ap())
        gbias = constp.tile([128, 2], F32, tag="gb")
        nc.sync.dma_start(out=gbias[:], in_=gb_d.ap())
        g0_s = constp.tile([128, TB], F32, tag="g0")
        g1_s = constp.tile([128, TB], F32, tag="g1")

        xT = xp.tile([128, KT_HID, S], BF, tag="xT")
        for kq in range(4):
            nc.sync.dma_start(out=xT[:, 4 * kq:4 * kq + 4, :],
                              in_=r3(xT_d, KT_HID)[:, 4 * kq:4 * kq + 4, :])

        # ---- weight prefetch ring: 4 rotating 16KB/partition chunks ----
        def ring_chunk(name):
            return wring.tile([128, 8192], BF, tag="w", name=name)

        def kview(ap, k, n):
            return ap.rearrange("p (k n) -> p k n", k=k, n=n)

        wt_qa = []
        for ck in range(3):
            chq = ring_chunk(f"qa{ck}")
            v = kview(chq, KT_HID, 512)
            nc.sync.dma_start(out=v[:], in_=r3(qa_d, KT_HID)[:, :, ts(ck, 512)])
            wt_qa.append(v)
        ch_kvl = ring_chunk("kvl")
        wt_kl = kview(ch_kvl, KT_HID, 512)
        nc.sync.dma_start(out=wt_kl[:], in_=r3(kvl_d, KT_HID))
        ch_kvp = ring_chunk("kvp")
        wt_kp = kview(ch_kvp[:, 0:4096], KT_HID, HPC * D_ROPE)
        nc.sync.dma_start(out=wt_kp[:], in_=r3(kvp_d, KT_HID))
        gwt = kview(ch_kvp[:, 4096:4128], KT_HID, 2)
        nc.sync.dma_start(out=gwt[:], in_=r3(gw_d, KT_HID))
        ch_w2a = ring_chunk("w2a")
        wqbn = kview(ch_w2a[:, 0:6144], KT_QR, HPC * D_NOPE)
        nc.sync.dma_start(out=wqbn[:], in_=r3(qbn_d, KT_QR))
        wkbn = kview(ch_w2a[:, 6144:8192], KT_KVR, HPC * D_NOPE)
        nc.sync.dma_start(out=wkbn[:], in_=r3(kbn_d, KT_KVR))
        ch_w2b = ring_chunk("w2b")
        wqbp = kview(ch_w2b[:, 0:3072], KT_QR, HPC * D_ROPE)
        nc.sync.dma_start(out=wqbp[:], in_=r3(qbp_d, KT_QR))
        wkbv = kview(ch_w2b[:, 3072:5120], KT_KVR, HPC * D_V)
        nc.sync.dma_start(out=wkbv[:], in_=r3(kbv_d, KT_KVR))
        ch_spq = ring_chunk("spq")
        wspq = kview(ch_spq, KT_HID, HPC * PD)
        nc.sync.dma_start(out=wspq[:], in_=r3(spq_d, KT_HID))
        ch_spk = ring_chunk("spk")
        wspk = kview(ch_spk, KT_HID, HPC * PD)
        nc.sync.dma_start(out=wspk[:], in_=r3(spk_d, KT_HID))
        ch_spv = ring_chunk("spv")
        wspv = kview(ch_spv, KT_HID, HPC * PD)
        nc.sync.dma_start(out=wspv[:], in_=r3(spv_d, KT_HID))
        ch_wo = ring_chunk("wo")
        wo = kview(ch_wo, KT_KVR, HID)
        nc.sync.dma_start(out=wo[:], in_=r3(ow_d, KT_KVR))
        ch_wspo = ring_chunk("wspo")
        wspo = kview(ch_wspo, KT_KVR, HID)
        nc.sync.dma_start(out=wspo[:], in_=r3(spo_d, KT_KVR))

        def rope_from_psum(ps, dst, nck, work):
            """Apply rope to a [128, 512] psum chunk holding 2 stacked
            64-dim pe heads; write bf16 to dst ([128,512] slice)."""
            rot = work.tile([128, 512], F32, tag="rot")
            nc.vector.tensor_scalar_mul(rot[0:32, :], ps[32:64, :], -1.0)
            nc.vector.tensor_copy(rot[32:64, :], ps[0:32, :])
            nc.vector.tensor_scalar_mul(rot[64:96, :], ps[96:128, :], -1.0)
            nc.vector.tensor_copy(rot[96:128, :], ps[64:96, :])
            t1 = work.tile([128, 512], F32, tag="t1")
            nc.vector.tensor_mul(t1[:], ps[:], cosT[:, ts(nck, 512)])
            nc.vector.tensor_mul(rot[:], rot[:], sinT[:, ts(nck, 512)])
            nc.vector.tensor_add(dst, t1[:], rot[:])

        def rms_scale_rows(ssq_ps, width, scale_k, dstT, mtiles, wk):
            """Given ssq row psum [1, width], scale dstT tiles in place by
            1/rms broadcast across partitions (via PE outer product)."""
            rms_row = small.tile([1, width], F32, tag="rmsrow", bufs=1)
            nc.scalar.activation(rms_row[:], ssq_ps[0:1, 0:width], Sqrt,
                                 bias=eps_t[0:1, :], scale=scale_k)
            inv_row = small.tile([1, width], F32, tag="invrow1", bufs=1)
            nc.vector.reciprocal(inv_row[:], rms_row[:])
            invb_row = small.tile([1, width], BF, tag="invrowb1", bufs=1)
            nc.vector.tensor_copy(invb_row[:], inv_row[:])
            for nck in range(width // 512):
                bcp = pt.tile([128, 512], F32, tag="pt")
                nc.tensor.matmul(bcp[:], lhsT=ones_row[:],
                                 rhs=invb_row[0:1, ts(nck, 512)],
                                 start=True, stop=True)
                bcs = wk.tile([128, 512], BF, tag="bcs")
                nc.any.tensor_copy(bcs[:], bcp[:])
                for m in range(mtiles):
                    nc.vector.tensor_mul(dstT[:, m, ts(nck, 512)],
                                         dstT[:, m, ts(nck, 512)], bcs[:])

        # k-major attention: scoresT[k,q] on PE, unnormalized exp
        # (|score| <= ||q||*||k||/sqrt(D) stays well inside f32 exp range
        # for this model), v-stationary ctx matmuls at N=512, denominators
        # via DVE tree-sum + GpSimd partition all-reduce.
        def attention(h, qh, qnT, knT, qpT, kpT, vv, voff, ctxT, is_main, awk):
            probsT = awk.tile([128, TB, 512], BF, tag="probsT")
            for kb in range(TB):
                ps = pp.tile([128, 512], F32, tag="pp")
                nc.tensor.matmul(ps[:], lhsT=knT[:, h, ts(kb, 128)],
                                 rhs=qnT[:, h, ts(qh, 512)],
                                 start=True, stop=not is_main)
                if is_main:
                    pb = (h % 2) * 64
                    nc.tensor.matmul(
                        ps[:],
                        lhsT=kpT[pb:pb + 64, h // 2, ts(kb, 128)],
                        rhs=qpT[pb:pb + 64, h // 2, ts(qh, 512)],
                        start=False, stop=True)
                nc.scalar.activation(probsT[:, kb, :], ps[:], Exp)
            tr = [awk.tile([128, 512], BF, tag=f"tr{i}", name=f"tr{i}")
                  for i in range(4)]
            for i in range(4):
                nc.vector.tensor_add(tr[i][:], probsT[:, 2 * i, :],
                                     probsT[:, 2 * i + 1, :])
            nc.vector.tensor_add(tr[0][:], tr[0][:], tr[1][:])
            nc.vector.tensor_add(tr[2][:], tr[2][:], tr[3][:])
            nc.vector.tensor_add(tr[0][:], tr[0][:], tr[2][:])
            ct = pt.tile([128, 512], F32, tag="pt")
            for kb in range(TB):
                nc.tensor.matmul(ct[:], lhsT=vv[:, kb, voff:voff + 128],
                                 rhs=probsT[:, kb, :],
                                 start=(kb == 0), stop=(kb == TB - 1))
            cts = awk.tile([128, 512], F32, tag="cts")
            nc.any.tensor_copy(cts[:], ct[:])
            ars = awk.tile([128, 512], F32, tag="ars")
            nc.gpsimd.partition_all_reduce(ars[:], tr[0][:], 128,
                                           bass_isa.ReduceOp.add)
            inv = awk.tile([128, 512], F32, tag="inv")
            nc.vector.reciprocal(inv[:], ars[:])
            nc.vector.tensor_mul(ctxT[:, h, ts(qh, 512)], cts[:], inv[:])

        ctxT_m = ctxp.tile([128, HPC, S], BF, tag="ctxm")
        ctxT_p = ctxp.tile([128, HPC, S], BF, tag="ctxp")

        # ================= stage 1+2 (scratch pools scoped) =================
        es_ain = ExitStack()
        ain = es_ain.enter_context(tc.tile_pool(name="ain_mla", bufs=1))
        qnopeT = ain.tile([128, HPC, S], BF, tag="qnopeT")
        qpeT = ain.tile([128, 2, S], BF, tag="qpeT")
        knopeT = ain.tile([128, HPC, S], BF, tag="knopeT")
        kpeT = ain.tile([128, 2, S], BF, tag="kpeT")
        v_s = ain.tile([128, TB, HPC * D_V], BF, tag="v")   # token-major

        es12 = ExitStack()
        wrope = es12.enter_context(tc.tile_pool(name="wrope", bufs=1))
        q2 = es12.enter_context(tc.tile_pool(name="q2", bufs=1))
        wk1 = es12.enter_context(tc.tile_pool(name="wk1", bufs=3))
        qmidT = q2.tile([128, KT_QR, S], BF, tag="qmidT")
        kvnT = q2.tile([128, KT_KVR, S], BF, tag="kvnT")

        # q_a: feature-major [QR, S]; rmsnorm stats via ACT square +
        # PE ones-reduction (batched after each weight chunk)
        ssq_q = psum1.tile([1, 1024], F32, tag="sm")
        for ck in range(3):
            wt = wt_qa[ck]
            sqs = []
            for mm4 in range(4):
                m = ck * 4 + mm4
                for nck in range(2):
                    ps = pp.tile([128, 512], F32, tag="pp")
                    for k in range(KT_HID):
                        nc.tensor.matmul(ps[:], lhsT=wt[:, k, ts(mm4, 128)],
                                         rhs=xT[:, k, ts(nck, 512)],
                                         start=(k == 0), stop=(k == KT_HID - 1))
                    nc.any.tensor_copy(qmidT[:, m, ts(nck, 512)], ps[:])
                    sq = wk1.tile([128, 512], BF, tag="sq", bufs=4)
                    nc.scalar.activation(sq[:], ps[:], Square)
                    sqs.append((m, nck, sq))
            for m, nck, sq in sqs:
                nc.tensor.matmul(ssq_q[0:1, ts(nck, 512)], lhsT=ones_col[:],
                                 rhs=sq[:], start=(m == 0), stop=(m == KT_QR - 1))
        rms_scale_rows(ssq_q, 1024, 1.0 / QR, qmidT, KT_QR, wk1)

        # kv_a lora part: feature-major [KVR, S]
        ssq_k = psum1.tile([1, 1024], F32, tag="sm")
        sqs_k = []
        for m in range(KT_KVR):
            for nck in range(2):
                ps = pp.tile([128, 512], F32, tag="pp")
                for k in range(KT_HID):
                    nc.tensor.matmul(ps[:], lhsT=wt_kl[:, k, ts(m, 128)],
                                     rhs=xT[:, k, ts(nck, 512)],
                                     start=(k == 0), stop=(k == KT_HID - 1))
                nc.any.tensor_copy(kvnT[:, m, ts(nck, 512)], ps[:])
                sq = wk1.tile([128, 512], BF, tag="sq", bufs=4)
                nc.scalar.activation(sq[:], ps[:], Square)
                sqs_k.append((m, nck, sq))
        for m, nck, sq in sqs_k:
            nc.tensor.matmul(ssq_k[0:1, ts(nck, 512)], lhsT=ones_col[:],
                             rhs=sq[:], start=(m == 0), stop=(m == KT_KVR - 1))
        rms_scale_rows(ssq_k, 1024, 1.0 / KVR, kvnT, KT_KVR, wk1)

        # kv_a pe part: feature-major (2 heads per M-tile) + rope
        for m in range(2):
            for nck in range(2):
                ps = pt.tile([128, 512], F32, tag="pt")
                for k in range(KT_HID):
                    nc.tensor.matmul(ps[:], lhsT=wt_kp[:, k, ts(m, 128)],
                                     rhs=xT[:, k, ts(nck, 512)],
                                     start=(k == 0), stop=(k == KT_HID - 1))
                rope_from_psum(ps, kpeT[:, m, ts(nck, 512)], nck, wrope)

        # gate (early, frees its ring chunk)
        for tb in range(TB):
            psg = pp.tile([128, 2], F32, tag="pp")
            for k in range(KT_HID):
                nc.tensor.matmul(psg[:], lhsT=xT[:, k, ts(tb, 128)],
                                 rhs=gwt[:, k, :],
                                 start=(k == 0), stop=(k == KT_HID - 1))
            glog = small.tile([128, 2], F32, tag="glog")
            nc.vector.tensor_add(glog[:], psg[:], gbias[:])
            gm = small.tile([128, 1], F32, tag="gm")
            nc.vector.reduce_max(gm[:], glog[:], axis=X)
            nc.vector.tensor_scalar_mul(gm[:], gm[:], -1.0)
            gexp = small.tile([128, 2], F32, tag="gexp")
            gsum = small.tile([128, 1], F32, tag="gsum")
            nc.scalar.activation(gexp[:], glog[:], Exp, bias=gm[:],
                                 accum_out=gsum[:])
            ginv = small.tile([128, 1], F32, tag="ginv")
            nc.vector.reciprocal(ginv[:], gsum[:])
            nc.vector.tensor_scalar_mul(g0_s[:, tb:tb + 1], gexp[:, 0:1], ginv[:])
            nc.vector.tensor_scalar_mul(g1_s[:, tb:tb + 1], gexp[:, 1:2], ginv[:])

        # ---------- Stage 2: b-projections ----------
        for h in range(HPC):
            for nck in range(2):
                ps = pt.tile([128, 512], F32, tag="pt")
                for k in range(KT_QR):
                    nc.tensor.matmul(ps[:], lhsT=wqbn[:, k, ts(h, 128)],
                                     rhs=qmidT[:, k, ts(nck, 512)],
                                     start=(k == 0), stop=(k == KT_QR - 1))
                nc.any.tensor_copy(qnopeT[:, h, ts(nck, 512)], ps[:])
        for m in range(2):
            for nck in range(2):
                ps = pt.tile([128, 512], F32, tag="pt")
                for k in range(KT_QR):
                    nc.tensor.matmul(ps[:], lhsT=wqbp[:, k, ts(m, 128)],
                                     rhs=qmidT[:, k, ts(nck, 512)],
                                     start=(k == 0), stop=(k == KT_QR - 1))
                rope_from_psum(ps, qpeT[:, m, ts(nck, 512)], nck, wrope)
        for h in range(HPC):
            for nck in range(2):
                ps = pt.tile([128, 512], F32, tag="pt")
                for k in range(KT_KVR):
                    nc.tensor.matmul(ps[:], lhsT=wkbn[:, k, ts(h, 128)],
                                     rhs=kvnT[:, k, ts(nck, 512)],
                                     start=(k == 0), stop=(k == KT_KVR - 1))
                nc.any.tensor_copy(knopeT[:, h, ts(nck, 512)], ps[:])
        for tb in range(TB):
            ps = pt.tile([128, 512], F32, tag="pt")
            for k in range(KT_KVR):
                nc.tensor.matmul(ps[:], lhsT=kvnT[:, k, ts(tb, 128)],
                                 rhs=wkbv[:, k, :],
                                 start=(k == 0), stop=(k == KT_KVR - 1))
            nc.any.tensor_copy(v_s[:, tb, :], ps[:])

        # stage 1/2 scratch released
        es12.close()

        # ---------- Stage 4a: MLA attention ----------
        with tc.tile_pool(name="awk", bufs=2) as awk:
            for h in range(HPC):
                for qh in range(2):
                    attention(h, qh, qnopeT, knopeT, qpeT, kpeT,
                              v_s, h * D_V, ctxT_m, True, awk)

        # MLA inputs released; pattern stage reuses that space
        es_ain.close()

        # ---------- Stage 3: pattern projections ----------
        with tc.tile_pool(name="ain_pat", bufs=1) as ainp:
            pqT = ainp.tile([128, HPC, S], BF, tag="pqT")
            pkT = ainp.tile([128, HPC, S], BF, tag="pkT")
            pv_s = ainp.tile([128, TB, HPC * PD], BF, tag="pv")

            for m in range(HPC):
                for nck in range(2):
                    ps = pt.tile([128, 512], F32, tag="pt")
                    for k in range(KT_HID):
                        nc.tensor.matmul(ps[:], lhsT=wspq[:, k, ts(m, 128)],
                                         rhs=xT[:, k, ts(nck, 512)],
                                         start=(k == 0), stop=(k == KT_HID - 1))
                    nc.any.tensor_copy(pqT[:, m, ts(nck, 512)], ps[:])
            for m in range(HPC):
                for nck in range(2):
                    ps = pt.tile([128, 512], F32, tag="pt")
                    for k in range(KT_HID):
                        nc.tensor.matmul(ps[:], lhsT=wspk[:, k, ts(m, 128)],
                                         rhs=xT[:, k, ts(nck, 512)],
                                         start=(k == 0), stop=(k == KT_HID - 1))
                    nc.any.tensor_copy(pkT[:, m, ts(nck, 512)], ps[:])
            for tb in range(TB):
                ps = pt.tile([128, 512], F32, tag="pt")
                for k in range(KT_HID):
                    nc.tensor.matmul(ps[:], lhsT=xT[:, k, ts(tb, 128)],
                                     rhs=wspv[:, k, :],
                                     start=(k == 0), stop=(k == KT_HID - 1))
                nc.any.tensor_copy(pv_s[:, tb, :], ps[:])

            # ---------- Stage 4b: pattern attention ----------
            with tc.tile_pool(name="awk2", bufs=2) as awk2:
                for h in range(HPC):
                    for qh in range(2):
                        attention(h, qh, pqT, pkT, None, None,
                                  pv_s, h * PD, ctxT_p, False, awk2)

        # ---------- Stage 5: output projections + gate combine ----------
        with tc.tile_pool(name="ow", bufs=2) as ow:
            for tb in range(TB):
                osb = ow.tile([128, HID], F32, tag="osb")
                for ck in range(4):
                    pm = pp.tile([128, 512], F32, tag="pp")
                    for k in range(KT_KVR):
                        nc.tensor.matmul(pm[:], lhsT=ctxT_m[:, k, ts(tb, 128)],
                                         rhs=wo[:, k, ts(ck, 512)],
                                         start=(k == 0), stop=(k == KT_KVR - 1))
                    pq2 = pp.tile([128, 512], F32, tag="pp")
                    for k in range(KT_KVR):
                        nc.tensor.matmul(pq2[:], lhsT=ctxT_p[:, k, ts(tb, 128)],
                                         rhs=wspo[:, k, ts(ck, 512)],
                                         start=(k == 0), stop=(k == KT_KVR - 1))
                    tmp = ow.tile([128, 512], F32, tag="tmp")
                    nc.vector.tensor_scalar_mul(tmp[:], pq2[:], g1_s[:, tb:tb + 1])
                    nc.vector.scalar_tensor_tensor(
                        osb[:, ts(ck, 512)], in0=pm[:], scalar=g0_s[:, tb:tb + 1],
                        in1=tmp[:], op0=MULT, op1=ADD)
                nc.gpsimd.dma_start(out=out_d[ts(tb, 128), :], in_=osb[:])

    nc.compile()
    return nc


def _rope_tables():
    inv_freq = 1.0 / (THETA ** (np.arange(0, D_ROPE, 2, dtype=np.float32) / D_ROPE))
    t = np.arange(S, dtype=np.float32)
    freqs = np.outer(t, inv_freq)                       # [S, 32]
    emb = np.concatenate([freqs, freqs], -1)            # [S, 64]
    cosT = np.cos(emb).T.astype(np.float32)             # [64, S]
    sinT = np.sin(emb).T.astype(np.float32)
    cos2T = np.ascontiguousarray(np.concatenate([cosT, cosT], 0))   # [128, S]
    sin2T = np.ascontiguousarray(np.concatenate([sinT, sinT], 0))
    return cos2T.astype(BF16), sin2T.astype(BF16)


def _prep_in_maps(hidden_states, q_a_w, q_a_ln_w, q_b_w, kv_a_w, kv_a_ln_w,
                  kv_b_w, o_w, sp_q_w, sp_k_w, sp_v_w, sp_o_w, gate_w, gate_b):
    def bf(x):
        return np.ascontiguousarray(x).astype(BF16)

    cos2T, sin2T = _rope_tables()
    qa_wT = bf(q_a_w.T)                                   # [HID, QR]
    kvl_wT = bf(kv_a_w[:KVR].T)                           # [HID, KVR]
    kv_a_pe = kv_a_w[KVR:].reshape(H, D_ROPE, HID)        # [H, 64, HID]

    qb = (q_b_w * q_a_ln_w[None, :]).reshape(H, D_Q, QR) * (D_Q ** -0.5)
    qb_nope = qb[:, :D_NOPE]                              # [H,128,QR]
    qb_pe = qb[:, D_NOPE:]                                # [H,64,QR]
    kvb = (kv_b_w * kv_a_ln_w[None, :]).reshape(H, D_NOPE + D_V, KVR)
    kb_nope = kvb[:, :D_NOPE]                             # [H,128,KVR]
    kb_v = kvb[:, D_NOPE:]                                # [H,128,KVR]
    o_wh = o_w.reshape(HID, H, D_V)                       # [HID,H,128]
    spq = (sp_q_w * (PD ** -0.5)).reshape(PH, PD, HID)
    spk = sp_k_w.reshape(PH, PD, HID)
    spv = sp_v_w.reshape(PH, PD, HID)
    spo = sp_o_w.reshape(HID, PH, PD)
    gate_wT = bf(gate_w.T)                                # [HID, 2]
    gate_bias = np.ascontiguousarray(
        np.broadcast_to(gate_b[None, :], (128, 2))).astype(np.float32)

    in_maps = []
    for c in range(NCORES):
        b, g = c // 4, c % 4
        hs = slice(4 * g, 4 * g + 4)
        m = {
            "xT": bf(hidden_states[b].T),
            "qa_wT": qa_wT,
            "qbn_wT": bf(qb_nope[hs].reshape(HPC * D_NOPE, QR).T),
            "qbp_wT": bf(qb_pe[hs].reshape(HPC * D_ROPE, QR).T),
            "kvl_wT": kvl_wT,
            "kvp_wT": bf(kv_a_pe[hs].reshape(HPC * D_ROPE, HID).T),
            "kbn_wT": bf(kb_nope[hs].reshape(HPC * D_NOPE, KVR).T),
            "kbv_wT": bf(kb_v[hs].reshape(HPC * D_V, KVR).T),
            "o_wT": bf(o_wh[:, hs].reshape(HID, HPC * D_V).T),
            "spq_wT": bf(spq[hs].reshape(HPC * PD, HID).T),
            "spk_wT": bf(spk[hs].reshape(HPC * PD, HID).T),
            "spv_wT": bf(spv[hs].reshape(HPC * PD, HID).T),
            "spo_wT": bf(spo[:, hs].reshape(HID, HPC * PD).T),
            "gate_wT": gate_wT,
            "gate_bias": gate_bias,
            "cos2T": cos2T,
            "sin2T": sin2T,
        }
        in_maps.append(m)
    return in_maps


def kernel(**inputs):
    global LAST_RESULT
    from concourse.bass_utils import run_bass_kernel_spmd

    inputs = {k: np.asarray(v) for k, v in inputs.items()}
    if "nc" not in _graph_cache:
        _graph_cache["nc"] = _build_graph()
    nc = _graph_cache["nc"]

    in_maps = _prep_in_maps(**inputs)
    res = run_bass_kernel_spmd(nc, in_maps, core_ids=list(range(NCORES)),
                               trace=TRACE, **RUN_KWARGS)
    LAST_RESULT = res
    out = np.zeros((B, S, HID), np.float32)
    for c in range(NCORES):
        out[c // 4] += res.results[c]["out"]
    return out


# revision 26
# speedup vs baseline: 1.2756x; 1.1996x over previous
"""ARCAttention (MLA + pattern-attention + gate) distributed Bass kernel for 8 TRN2 NeuronCores.

Sharding: data-parallel over batch (B=2) x tensor-parallel over heads (4 head-groups).
Core c handles batch (c // 4), heads [4*(c%4) .. 4*(c%4)+4) of both the MLA path and the
pattern path. The low-rank a-projections (q_a, kv_a lora) and the gate are replicated
within a batch group. Each core emits a partial (already gate-weighted) output
[S, HID]; the host sums the 4 partials per batch. No device collectives.

All matmuls run in bf16 (f32 PSUM accumulation); softmax/rmsnorm statistics in f32.
Weight preprocessing (transposes, ln-weight folding, scale folding, rope tables) is
done on host in numpy and shipped per-core via in_maps.
"""

import numpy as np
import ml_dtypes

# ---- model config (hardcoded from the problem spec) ----
B, S, HID = 2, 1024, 2048
H = 16
D_NOPE, D_ROPE, D_V = 128, 64, 128
D_Q = D_NOPE + D_ROPE            # 192
QR, KVR = 1536, 512
PH, PD = 16, 128
THETA, EPS = 10000.0, 1e-6
NCORES = 8
HPC = 4                          # heads per core
TB = S // 128                    # 8 token blocks
KT_HID = HID // 128              # 16
KT_QR = QR // 128                # 12
KT_KVR = KVR // 128              # 4

BF16 = ml_dtypes.bfloat16

# knobs for test harness
TRACE = False
RUN_KWARGS = {}
LAST_RESULT = None

_graph_cache = {}


def _build_graph():
    from contextlib import ExitStack
    import concourse.bass as bass
    import concourse.mybir as mybir
    import concourse.tile as tile
    from concourse import bacc, bass_isa

    BF = mybir.dt.bfloat16
    F32 = mybir.dt.float32
    Exp = mybir.ActivationFunctionType.Exp
    Square = mybir.ActivationFunctionType.Square
    Sqrt = mybir.ActivationFunctionType.Sqrt
    MULT = mybir.AluOpType.mult
    ADD = mybir.AluOpType.add
    X = mybir.AxisListType.X
    ts = bass.ts

    nc = bacc.Bacc("TRN2", target_bir_lowering=False, debug=False,
                   num_devices=NCORES)

    def din(name, shape, dt=BF):
        return nc.declare_dram_parameter(name, list(shape), dt, isOutput=False)

    xT_d = din("xT", [HID, S])
    qa_d = din("qa_wT", [HID, QR])
    qbn_d = din("qbn_wT", [QR, HPC * D_NOPE])
    qbp_d = din("qbp_wT", [QR, HPC * D_ROPE])
    kvl_d = din("kvl_wT", [HID, KVR])
    kvp_d = din("kvp_wT", [HID, HPC * D_ROPE])
    kbn_d = din("kbn_wT", [KVR, HPC * D_NOPE])
    kbv_d = din("kbv_wT", [KVR, HPC * D_V])
    ow_d = din("o_wT", [HPC * D_V, HID])
    spq_d = din("spq_wT", [HID, HPC * PD])
    spk_d = din("spk_wT", [HID, HPC * PD])
    spv_d = din("spv_wT", [HID, HPC * PD])
    spo_d = din("spo_wT", [HPC * PD, HID])
    gw_d = din("gate_wT", [HID, 2])
    gb_d = din("gate_bias", [128, 2], F32)
    cos_d = din("cos2T", [128, S])
    sin_d = din("sin2T", [128, S])
    out_d = nc.declare_dram_parameter("out", [S, HID], F32, isOutput=True)

    def r3(dram, kt):
        # [kt*128, N] dram tensor viewed as [128, kt, N] for SBUF tiling
        return dram.ap().rearrange("(k p) n -> p k n", p=128, k=kt)

    es = ExitStack()
    with tile.TileContext(nc) as tc, es:
        constp = es.enter_context(tc.tile_pool(name="const", bufs=1))
        small = es.enter_context(tc.tile_pool(name="small", bufs=4))
        pp = es.enter_context(tc.tile_pool(name="pp", bufs=4, space="PSUM"))
        pt = es.enter_context(tc.tile_pool(name="pt", bufs=2, space="PSUM"))
        psum1 = es.enter_context(tc.tile_pool(name="psum1", bufs=1, space="PSUM"))
        wring = es.enter_context(tc.tile_pool(name="wring", bufs=3))
        xp = es.enter_context(tc.tile_pool(name="xp", bufs=1))
        ctxp = es.enter_context(tc.tile_pool(name="ctxp", bufs=1))

        eps_t = constp.tile([128, 1], F32, tag="eps")
        nc.vector.memset(eps_t[:], EPS)
        ones_col = constp.tile([128, 1], BF, tag="ones_col")
        nc.vector.memset(ones_col[:], 1.0)
        ones_row = constp.tile([1, 128], BF, tag="ones_row")
        nc.vector.memset(ones_row[:], 1.0)
        cosT = constp.tile([128, S], BF, tag="cos")
        sinT = constp.tile([128, S], BF, tag="sin")
        nc.sync.dma_start(out=cosT[:], in_=cos_d.ap())
        nc.sync.dma_start(out=sinT[:], in_=sin_d.ap())
        gbias = constp.tile([128, 2], F32, tag="gb")
        nc.sync.dma_start(out=gbias[:], in_=gb_d.ap())
        g0_s = constp.tile([128, TB], F32, tag="g0")
        g1_s = constp.tile([128, TB], F32, tag="g1")

        xT = xp.tile([128, KT_HID, S], BF, tag="xT")
        for kq in range(4):
            nc.scalar.dma_start(out=xT[:, 4 * kq:4 * kq + 4, :],
                                in_=r3(xT_d, KT_HID)[:, 4 * kq:4 * kq + 4, :])

        # ---- weight prefetch ring: 4 rotating 16KB/partition chunks ----
        def ring_chunk(name):
            return wring.tile([128, 8192], BF, tag="w", name=name)

        def kview(ap, k, n):
            return ap.rearrange("p (k n) -> p k n", k=k, n=n)

        wt_qa = []
        for ck in range(3):
            chq = ring_chunk(f"qa{ck}")
            v = kview(chq, KT_HID, 512)
            nc.sync.dma_start(out=v[:], in_=r3(qa_d, KT_HID)[:, :, ts(ck, 512)])
            wt_qa.append(v)
        ch_kvl = ring_chunk("kvl")
        wt_kl = kview(ch_kvl, KT_HID, 512)
        nc.sync.dma_start(out=wt_kl[:], in_=r3(kvl_d, KT_HID))
        ch_kvp = ring_chunk("kvp")
        wt_kp = kview(ch_kvp[:, 0:4096], KT_HID, HPC * D_ROPE)
        nc.sync.dma_start(out=wt_kp[:], in_=r3(kvp_d, KT_HID))
        gwt = kview(ch_kvp[:, 4096:4128], KT_HID, 2)
        nc.sync.dma_start(out=gwt[:], in_=r3(gw_d, KT_HID))
        ch_w2a = ring_chunk("w2a")
        wqbn = kview(ch_w2a[:, 0:6144], KT_QR, HPC * D_NOPE)
        nc.sync.dma_start(out=wqbn[:], in_=r3(qbn_d, KT_QR))
        wkbn = kview(ch_w2a[:, 6144:8192], KT_KVR, HPC * D_NOPE)
        nc.sync.dma_start(out=wkbn[:], in_=r3(kbn_d, KT_KVR))
        ch_w2b = ring_chunk("w2b")
        wqbp = kview(ch_w2b[:, 0:3072], KT_QR, HPC * D_ROPE)
        nc.sync.dma_start(out=wqbp[:], in_=r3(qbp_d, KT_QR))
        wkbv = kview(ch_w2b[:, 3072:5120], KT_KVR, HPC * D_V)
        nc.sync.dma_start(out=wkbv[:], in_=r3(kbv_d, KT_KVR))
        ch_spq = ring_chunk("spq")
        wspq = kview(ch_spq, KT_HID, HPC * PD)
        nc.sync.dma_start(out=wspq[:], in_=r3(spq_d, KT_HID))
        ch_spk = ring_chunk("spk")
        wspk = kview(ch_spk, KT_HID, HPC * PD)
        nc.sync.dma_start(out=wspk[:], in_=r3(spk_d, KT_HID))
        ch_spv = ring_chunk("spv")
        wspv = kview(ch_spv, KT_HID, HPC * PD)
        nc.sync.dma_start(out=wspv[:], in_=r3(spv_d, KT_HID))
        ch_wo = ring_chunk("wo")
        wo = kview(ch_wo, KT_KVR, HID)
        nc.sync.dma_start(out=wo[:], in_=r3(ow_d, KT_KVR))
        ch_wspo = ring_chunk("wspo")
        wspo = kview(ch_wspo, KT_KVR, HID)
        nc.sync.dma_start(out=wspo[:], in_=r3(spo_d, KT_KVR))

        def rope_from_psum(ps, dst, nck, work):
            """Apply rope to a [128, 512] psum chunk holding 2 stacked
            64-dim pe heads; write bf16 to dst ([128,512] slice)."""
            rot = work.tile([128, 512], F32, tag="rot")
            nc.vector.tensor_scalar_mul(rot[0:32, :], ps[32:64, :], -1.0)
            nc.vector.tensor_copy(rot[32:64, :], ps[0:32, :])
            nc.vector.tensor_scalar_mul(rot[64:96, :], ps[96:128, :], -1.0)
            nc.vector.tensor_copy(rot[96:128, :], ps[64:96, :])
            t1 = work.tile([128, 512], F32, tag="t1")
            nc.vector.tensor_mul(t1[:], ps[:], cosT[:, ts(nck, 512)])
            nc.vector.tensor_mul(rot[:], rot[:], sinT[:, ts(nck, 512)])
            nc.vector.tensor_add(dst, t1[:], rot[:])

        def rms_scale_rows(ssq_ps, width, scale_k, dstT, mtiles, wk):
            """Given ssq row psum [1, width], scale dstT tiles in place by
            1/rms broadcast across partitions (via PE outer product)."""
            rms_row = small.tile([1, width], F32, tag="rmsrow", bufs=1)
            nc.scalar.activation(rms_row[:], ssq_ps[0:1, 0:width], Sqrt,
                                 bias=eps_t[0:1, :], scale=scale_k)
            rmsb_row = small.tile([1, width], BF, tag="rmsb1", bufs=1)
            nc.vector.tensor_copy(rmsb_row[:], rms_row[:])
            for nck in range(width // 512):
                bcp = pt.tile([128, 512], F32, tag="pt")
                nc.tensor.matmul(bcp[:], lhsT=ones_row[:],
                                 rhs=rmsb_row[0:1, ts(nck, 512)],
                                 start=True, stop=True)
                bcs = wk.tile([128, 512], F32, tag="bcs")
                nc.vector.reciprocal(bcs[:], bcp[:])
                for m in range(mtiles):
                    nc.vector.tensor_mul(dstT[:, m, ts(nck, 512)],
                                         dstT[:, m, ts(nck, 512)], bcs[:])

        # k-major attention: scoresT[k,q] on PE, unnormalized exp
        # (|score| <= ||q||*||k||/sqrt(D) stays well inside f32 exp range
        # for this model), v-stationary ctx matmuls at N=512, denominators
        # via DVE tree-sum + GpSimd partition all-reduce.
        def attention(h, qh, qnT, knT, qpT, kpT, vv, voff, ctxT, is_main, awk):
            probsT = awk.tile([128, TB, 512], BF, tag="probsT")
            for kb in range(TB):
                ps = pp.tile([128, 512], F32, tag="pp")
                nc.tensor.matmul(ps[:], lhsT=knT[:, h, ts(kb, 128)],
                                 rhs=qnT[:, h, ts(qh, 512)],
                                 start=True, stop=not is_main)
                if is_main:
                    pb = (h % 2) * 64
                    nc.tensor.matmul(
                        ps[:],
                        lhsT=kpT[pb:pb + 64, h // 2, ts(kb, 128)],
                        rhs=qpT[pb:pb + 64, h // 2, ts(qh, 512)],
                        start=False, stop=True)
                nc.scalar.activation(probsT[:, kb, :], ps[:], Exp)
            tr = [awk.tile([128, 512], BF, tag=f"tr{i}", name=f"tr{i}",
                           bufs=(4 if i == 0 else 2)) for i in range(4)]
            for i in range(4):
                nc.vector.tensor_add(tr[i][:], probsT[:, 2 * i, :],
                                     probsT[:, 2 * i + 1, :])
            nc.vector.tensor_add(tr[0][:], tr[0][:], tr[1][:])
            nc.vector.tensor_add(tr[2][:], tr[2][:], tr[3][:])
            nc.vector.tensor_add(tr[0][:], tr[0][:], tr[2][:])
            ct = pt.tile([128, 512], F32, tag="pt")
            for kb in range(TB):
                nc.tensor.matmul(ct[:], lhsT=vv[:, kb, voff:voff + 128],
                                 rhs=probsT[:, kb, :],
                                 start=(kb == 0), stop=(kb == TB - 1))
            cts = awk.tile([128, 512], F32, tag="cts", bufs=3)
            nc.any.tensor_copy(cts[:], ct[:])
            ars = awk.tile([128, 512], F32, tag="ars", bufs=3)
            nc.gpsimd.partition_all_reduce(ars[:], tr[0][:], 128,
                                           bass_isa.ReduceOp.add)
            inv = awk.tile([128, 512], F32, tag="inv", bufs=3)
            nc.vector.reciprocal(inv[:], ars[:])
            nc.vector.tensor_mul(ctxT[:, h, ts(qh, 512)], cts[:], inv[:])

        ctxT_m = ctxp.tile([128, HPC, S], BF, tag="ctxm")
        ctxT_p = ctxp.tile([128, HPC, S], BF, tag="ctxp")

        # ================= stage 1+2 (scratch pools scoped) =================
        es_ain = ExitStack()
        ain = es_ain.enter_context(tc.tile_pool(name="ain_mla", bufs=1))
        qnopeT = ain.tile([128, HPC, S], BF, tag="qnopeT")
        qpeT = ain.tile([128, 2, S], BF, tag="qpeT")
        knopeT = ain.tile([128, HPC, S], BF, tag="knopeT")
        kpeT = ain.tile([128, 2, S], BF, tag="kpeT")
        v_s = ain.tile([128, TB, HPC * D_V], BF, tag="v")   # token-major

        es12 = ExitStack()
        wrope = es12.enter_context(tc.tile_pool(name="wrope", bufs=1))
        q2 = es12.enter_context(tc.tile_pool(name="q2", bufs=1))
        wk1 = es12.enter_context(tc.tile_pool(name="wk1", bufs=3))
        qmidT = q2.tile([128, KT_QR, S], BF, tag="qmidT")
        kvnT = q2.tile([128, KT_KVR, S], BF, tag="kvnT")

        # q_a: feature-major [QR, S]; rmsnorm stats via ACT square +
        # PE ones-reduction (batched after each weight chunk)
        ssq_q = psum1.tile([1, 1024], F32, tag="sm")
        for ck in range(3):
            wt = wt_qa[ck]
            sqs = []
            for mm4 in range(4):
                m = ck * 4 + mm4
                for nck in range(2):
                    ps = pp.tile([128, 512], F32, tag="pp")
                    for k in range(KT_HID):
                        nc.tensor.matmul(ps[:], lhsT=wt[:, k, ts(mm4, 128)],
                                         rhs=xT[:, k, ts(nck, 512)],
                                         start=(k == 0), stop=(k == KT_HID - 1))
                    nc.any.tensor_copy(qmidT[:, m, ts(nck, 512)], ps[:])
                    sq = wk1.tile([128, 512], BF, tag="sq", bufs=4)
                    nc.scalar.activation(sq[:], ps[:], Square)
                    sqs.append((m, nck, sq))
            for m, nck, sq in sqs:
                nc.tensor.matmul(ssq_q[0:1, ts(nck, 512)], lhsT=ones_col[:],
                                 rhs=sq[:], start=(m == 0), stop=(m == KT_QR - 1))
        rms_scale_rows(ssq_q, 1024, 1.0 / QR, qmidT, KT_QR, wk1)

        # kv_a lora part: feature-major [KVR, S]
        ssq_k = psum1.tile([1, 1024], F32, tag="sm")
        sqs_k = []
        for m in range(KT_KVR):
            for nck in range(2):
                ps = pp.tile([128, 512], F32, tag="pp")
                for k in range(KT_HID):
                    nc.tensor.matmul(ps[:], lhsT=wt_kl[:, k, ts(m, 128)],
                                     rhs=xT[:, k, ts(nck, 512)],
                                     start=(k == 0), stop=(k == KT_HID - 1))
                nc.any.tensor_copy(kvnT[:, m, ts(nck, 512)], ps[:])
                sq = wk1.tile([128, 512], BF, tag="sq", bufs=4)
                nc.scalar.activation(sq[:], ps[:], Square)
                sqs_k.append((m, nck, sq))
        for m, nck, sq in sqs_k:
            nc.tensor.matmul(ssq_k[0:1, ts(nck, 512)], lhsT=ones_col[:],
                             rhs=sq[:], start=(m == 0), stop=(m == KT_KVR - 1))
        rms_scale_rows(ssq_k, 1024, 1.0 / KVR, kvnT, KT_KVR, wk1)

        # kv_a pe part: feature-major (2 heads per M-tile) + rope
        for m in range(2):
            for nck in range(2):
                ps = pt.tile([128, 512], F32, tag="pt")
                for k in range(KT_HID):
                    nc.tensor.matmul(ps[:], lhsT=wt_kp[:, k, ts(m, 128)],
                                     rhs=xT[:, k, ts(nck, 512)],
                                     start=(k == 0), stop=(k == KT_HID - 1))
                rope_from_psum(ps, kpeT[:, m, ts(nck, 512)], nck, wrope)

        # gate (early, frees its ring chunk)
        for tb in range(TB):
            psg = pp.tile([128, 2], F32, tag="pp")
            for k in range(KT_HID):
                nc.tensor.matmul(psg[:], lhsT=xT[:, k, ts(tb, 128)],
                                 rhs=gwt[:, k, :],
                                 start=(k == 0), stop=(k == KT_HID - 1))
            glog = small.tile([128, 2], F32, tag="glog")
            nc.vector.tensor_add(glog[:], psg[:], gbias[:])
            gm = small.tile([128, 1], F32, tag="gm")
            nc.vector.reduce_max(gm[:], glog[:], axis=X)
            nc.vector.tensor_scalar_mul(gm[:], gm[:], -1.0)
            gexp = small.tile([128, 2], F32, tag="gexp")
            gsum = small.tile([128, 1], F32, tag="gsum")
            nc.scalar.activation(gexp[:], glog[:], Exp, bias=gm[:],
                                 accum_out=gsum[:])
            ginv = small.tile([128, 1], F32, tag="ginv")
            nc.vector.reciprocal(ginv[:], gsum[:])
            nc.vector.tensor_scalar_mul(g0_s[:, tb:tb + 1], gexp[:, 0:1], ginv[:])
            nc.vector.tensor_scalar_mul(g1_s[:, tb:tb + 1], gexp[:, 1:2], ginv[:])

        # ---------- Stage 2: b-projections ----------
        for h in range(HPC):
            for nck in range(2):
                ps = pt.tile([128, 512], F32, tag="pt")
                for k in range(KT_QR):
                    nc.tensor.matmul(ps[:], lhsT=wqbn[:, k, ts(h, 128)],
                                     rhs=qmidT[:, k, ts(nck, 512)],
                                     start=(k == 0), stop=(k == KT_QR - 1))
                nc.any.tensor_copy(qnopeT[:, h, ts(nck, 512)], ps[:])
        for m in range(2):
            for nck in range(2):
                ps = pt.tile([128, 512], F32, tag="pt")
                for k in range(KT_QR):
                    nc.tensor.matmul(ps[:], lhsT=wqbp[:, k, ts(m, 128)],
                                     rhs=qmidT[:, k, ts(nck, 512)],
                                     start=(k == 0), stop=(k == KT_QR - 1))
                rope_from_psum(ps, qpeT[:, m, ts(nck, 512)], nck, wrope)
        for h in range(HPC):
            for nck in range(2):
                ps = pt.tile([128, 512], F32, tag="pt")
                for k in range(KT_KVR):
                    nc.tensor.matmul(ps[:], lhsT=wkbn[:, k, ts(h, 128)],
                                     rhs=kvnT[:, k, ts(nck, 512)],
                                     start=(k == 0), stop=(k == KT_KVR - 1))
                nc.any.tensor_copy(knopeT[:, h, ts(nck, 512)], ps[:])
        for tb in range(TB):
            ps = pt.tile([128, 512], F32, tag="pt")
            for k in range(KT_KVR):
                nc.tensor.matmul(ps[:], lhsT=kvnT[:, k, ts(tb, 128)],
                                 rhs=wkbv[:, k, :],
                                 start=(k == 0), stop=(k == KT_KVR - 1))
            nc.any.tensor_copy(v_s[:, tb, :], ps[:])

        # stage 1/2 scratch released
        es12.close()

        # ---------- Stage 4a: MLA attention ----------
        with tc.tile_pool(name="awk", bufs=2) as awk:
            for h in range(HPC):
                for qh in range(2):
                    attention(h, qh, qnopeT, knopeT, qpeT, kpeT,
                              v_s, h * D_V, ctxT_m, True, awk)

        # MLA inputs released; pattern stage reuses that space
        es_ain.close()

        # ---------- Stage 3: pattern projections ----------
        with tc.tile_pool(name="ain_pat", bufs=1) as ainp:
            pqT = ainp.tile([128, HPC, S], BF, tag="pqT")
            pkT = ainp.tile([128, HPC, S], BF, tag="pkT")
            pv_s = ainp.tile([128, TB, HPC * PD], BF, tag="pv")

            for m in range(HPC):
                for nck in range(2):
                    ps = pt.tile([128, 512], F32, tag="pt")
                    for k in range(KT_HID):
                        nc.tensor.matmul(ps[:], lhsT=wspq[:, k, ts(m, 128)],
                                         rhs=xT[:, k, ts(nck, 512)],
                                         start=(k == 0), stop=(k == KT_HID - 1))
                    nc.any.tensor_copy(pqT[:, m, ts(nck, 512)], ps[:])
            for m in range(HPC):
                for nck in range(2):
                    ps = pt.tile([128, 512], F32, tag="pt")
                    for k in range(KT_HID):
                        nc.tensor.matmul(ps[:], lhsT=wspk[:, k, ts(m, 128)],
                                         rhs=xT[:, k, ts(nck, 512)],
                                         start=(k == 0), stop=(k == KT_HID - 1))
                    nc.any.tensor_copy(pkT[:, m, ts(nck, 512)], ps[:])
            for tb in range(TB):
                ps = pt.tile([128, 512], F32, tag="pt")
                for k in range(KT_HID):
                    nc.tensor.matmul(ps[:], lhsT=xT[:, k, ts(tb, 128)],
                                     rhs=wspv[:, k, :],
                                     start=(k == 0), stop=(k == KT_HID - 1))
                nc.any.tensor_copy(pv_s[:, tb, :], ps[:])

            # ---------- Stage 4b: pattern attention ----------
            with tc.tile_pool(name="awk2", bufs=2) as awk2:
                for h in range(HPC):
                    for qh in range(2):
                        attention(h, qh, pqT, pkT, None, None,
                                  pv_s, h * PD, ctxT_p, False, awk2)

        # ---------- Stage 5: output projections + gate combine ----------
        with tc.tile_pool(name="ow", bufs=2) as ow:
            pm_sbs = {}
            for tb in range(TB):
                osb = ow.tile([128, HID], F32, tag="osb")
                for ck in range(4):
                    pm = pp.tile([128, 512], F32, tag="pp")
                    for k in range(KT_KVR):
                        nc.tensor.matmul(pm[:], lhsT=ctxT_m[:, k, ts(tb, 128)],
                                         rhs=wo[:, k, ts(ck, 512)],
                                         start=(k == 0), stop=(k == KT_KVR - 1))
                    pm_sb = ow.tile([128, 512], F32, tag="pmsb", bufs=8,
                                    name=f"pmsb{tb}_{ck}")
                    nc.any.tensor_copy(pm_sb[:], pm[:])
                    pm_sbs[(tb, ck)] = pm_sb
                    pq2 = pp.tile([128, 512], F32, tag="pp")
                    for k in range(KT_KVR):
                        nc.tensor.matmul(pq2[:], lhsT=ctxT_p[:, k, ts(tb, 128)],
                                         rhs=wspo[:, k, ts(ck, 512)],
                                         start=(k == 0), stop=(k == KT_KVR - 1))
                    tmp = ow.tile([128, 512], F32, tag="tmp")
                    nc.vector.tensor_scalar_mul(tmp[:], pq2[:], g1_s[:, tb:tb + 1])
                    nc.vector.scalar_tensor_tensor(
                        osb[:, ts(ck, 512)], in0=pm_sbs[(tb, ck)][:],
                        scalar=g0_s[:, tb:tb + 1],
                        in1=tmp[:], op0=MULT, op1=ADD)
                nc.gpsimd.dma_start(out=out_d[ts(tb, 128), :], in_=osb[:])

    nc.compile()
    return nc


def _rope_tables():
    inv_freq = 1.0 / (THETA ** (np.arange(0, D_ROPE, 2, dtype=np.float32) / D_ROPE))
    t = np.arange(S, dtype=np.float32)
    freqs = np.outer(t, inv_freq)                       # [S, 32]
    emb = np.concatenate([freqs, freqs], -1)            # [S, 64]
    cosT = np.cos(emb).T.astype(np.float32)             # [64, S]
    sinT = np.sin(emb).T.astype(np.float32)
    cos2T = np.ascontiguousarray(np.concatenate([cosT, cosT], 0))   # [128, S]
    sin2T = np.ascontiguousarray(np.concatenate([sinT, sinT], 0))
    return cos2T.astype(BF16), sin2T.astype(BF16)


def _prep_in_maps(hidden_states, q_a_w, q_a_ln_w, q_b_w, kv_a_w, kv_a_ln_w,
                  kv_b_w, o_w, sp_q_w, sp_k_w, sp_v_w, sp_o_w, gate_w, gate_b):
    def bf(x):
        return np.ascontiguousarray(x).astype(BF16)

    cos2T, sin2T = _rope_tables()
    qa_wT = bf(q_a_w.T)                                   # [HID, QR]
    kvl_wT = bf(kv_a_w[:KVR].T)                           # [HID, KVR]
    kv_a_pe = kv_a_w[KVR:].reshape(H, D_ROPE, HID)        # [H, 64, HID]

    qb = (q_b_w * q_a_ln_w[None, :]).reshape(H, D_Q, QR) * (D_Q ** -0.5)
    qb_nope = qb[:, :D_NOPE]                              # [H,128,QR]
    qb_pe = qb[:, D_NOPE:]                                # [H,64,QR]
    kvb = (kv_b_w * kv_a_ln_w[None, :]).reshape(H, D_NOPE + D_V, KVR)
    kb_nope = kvb[:, :D_NOPE]                             # [H,128,KVR]
    kb_v = kvb[:, D_NOPE:]                                # [H,128,KVR]
    o_wh = o_w.reshape(HID, H, D_V)                       # [HID,H,128]
    spq = (sp_q_w * (PD ** -0.5)).reshape(PH, PD, HID)
    spk = sp_k_w.reshape(PH, PD, HID)
    spv = sp_v_w.reshape(PH, PD, HID)
    spo = sp_o_w.reshape(HID, PH, PD)
    gate_wT = bf(gate_w.T)                                # [HID, 2]
    gate_bias = np.ascontiguousarray(
        np.broadcast_to(gate_b[None, :], (128, 2))).astype(np.float32)

    in_maps = []
    for c in range(NCORES):
        b, g = c // 4, c % 4
        hs = slice(4 * g, 4 * g + 4)
        m = {
            "xT": bf(hidden_states[b].T),
            "qa_wT": qa_wT,
            "qbn_wT": bf(qb_nope[hs].reshape(HPC * D_NOPE, QR).T),
            "qbp_wT": bf(qb_pe[hs].reshape(HPC * D_ROPE, QR).T),
            "kvl_wT": kvl_wT,
            "kvp_wT": bf(kv_a_pe[hs].reshape(HPC * D_ROPE, HID).T),
            "kbn_wT": bf(kb_nope[hs].reshape(HPC * D_NOPE, KVR).T),
            "kbv_wT": bf(kb_v[hs].reshape(HPC * D_V, KVR).T),
            "o_wT": bf(o_wh[:, hs].reshape(HID, HPC * D_V).T),
            "spq_wT": bf(spq[hs].reshape(HPC * PD, HID).T),
            "spk_wT": bf(spk[hs].reshape(HPC * PD, HID).T),
            "spv_wT": bf(spv[hs].reshape(HPC * PD, HID).T),
            "spo_wT": bf(spo[:, hs].reshape(HID, HPC * PD).T),
            "gate_wT": gate_wT,
            "gate_bias": gate_bias,
            "cos2T": cos2T,
            "sin2T": sin2T,
        }
        in_maps.append(m)
    return in_maps


def kernel(**inputs):
    global LAST_RESULT
    from concourse.bass_utils import run_bass_kernel_spmd

    inputs = {k: np.asarray(v) for k, v in inputs.items()}
    if "nc" not in _graph_cache:
        _graph_cache["nc"] = _build_graph()
    nc = _graph_cache["nc"]

    in_maps = _prep_in_maps(**inputs)
    res = run_bass_kernel_spmd(nc, in_maps, core_ids=list(range(NCORES)),
                               trace=TRACE, **RUN_KWARGS)
    LAST_RESULT = res
    out = np.zeros((B, S, HID), np.float32)
    for c in range(NCORES):
        out[c // 4] += res.results[c]["out"]
    return out
